# revision 21
# baseline (speedup 1.0000x reference)
"""Trainium2 Bass kernel for causal multi-head attention (dense transformer block).

Problem (hardcoded): x [2, 2048, 1024], 16 heads x 64 dh, causal attention,
fp32 I/O. Sharding: 8 cores = 2 batches x 4 head-groups. Each core computes 4
heads for one batch plus a partial output projection [2048, 1024] (bf16); the
host sums the 4 partials per batch and adds b_O.

Everything on-device is computed in "transposed" orientation so no transposes
are needed anywhere:
  x^T (host-pretransposed)  ->  Q^T, K^T [dh, s] and V [s, dh] via matmuls
  S^T[k, q] = K Q^T         ->  P^T = exp(S^T / 8) (causal-masked pre-exp)
  Z^T[dh, q] = V^T P^T      ->  normalized by column sums (ones-matmul)
  O[s, :]   = (Z^T)^T W_O   (Z^T is directly the lhsT of the O-projection)

Heads are processed in pairs: QK^T packs 2 heads in row-groups (0-63 / 64-127)
of the PE array, PV packs 2 heads in column-groups -- both run concurrently.

v2 schedule: the whole kernel is software-pipelined in emission order so the
scalar engine (softmax exp, the per-core floor at ~58us) overlaps all other
work:
  warmup (PE HAM + ACT table) during the input DMA wait
  for ch: qk_proj(pair0, ch) + v_proj(4 tiles); attention(pair0, qb=ch)
  for ch: qk_proj(pair1, ch); attention(pair1, qb=ch); O-proj chunks
PSUM: scores 2x2 banks, zps 1, dnb 1, proj/O 2 = 8 banks.
"""

import os
from contextlib import ExitStack

import numpy as np

import concourse.tile as tile
from concourse import bacc, mybir
from concourse.bass_utils import run_bass_kernel_spmd

# problem constants
B, S, DM, H, DH = 2, 2048, 1024, 16, 64
P = 128          # partitions
QB = 512         # q block (matmul moving free dim)
NKT = S // P     # 16 k tiles
NQB = S // QB    # 4 q blocks
NDM = DM // P    # 8 d_model tiles
HPC = 4          # heads per core
NCORES = 8
NWARM = 9        # PE warmup matmuls (~3.5us to beat the HAM cold clock)

F32 = mybir.dt.float32
BF16 = mybir.dt.bfloat16
FP8 = mybir.dt.float8e4

# fp8 DoubleRow Q/K projections: W_Q/W_K and x are quantized to fp8e4 on the
# host (weights pre-scaled by 64 so they clear the fp8 subnormal range; the
# 1/64^2 un-scale is folded into the softmax exp scale). Q/K quantization
# noise is random across the contraction and averages out through the
# softmax, unlike V/W_O noise which lands directly in the output.
QK_FP8 = os.environ.get("ATTN_QK_FP8", "1") == "1"
WSCALE = 64.0

_PROGRAM_CACHE = {}
LAST_RESULTS = None  # BassKernelResults of the most recent run (for test.py)


def _mm(nc, out, lhsT, rhs, start, stop, skip=False):
    # skip_group_check: the sim's psum-group tracker doesn't distinguish
    # partition ranges; our concurrent groups in one bank are partition-disjoint
    # (rows 0-63 vs 64-127), which the per-partition zeroing model handles.
    return nc.tensor.matmul(
        out, lhsT, rhs, start=start, stop=stop, skip_group_check=skip
    )


def _chain(insts):
    """Ordering-only PE edges so matmuls alternating between row/column
    groups stay adjacent and run concurrently on the array."""
    from concourse.tile import add_dep_helper

    for a, b in zip(insts[1:], insts):
        add_dep_helper(a.ins, b.ins, sync=False, reason="pack-pair order")


def build_program(qk_fp8=QK_FP8):
    """Build the single-core SPMD Bass program (same program on all 8 cores)."""
    if qk_fp8 in _PROGRAM_CACHE:
        return _PROGRAM_CACHE[qk_fp8]

    nc = bacc.Bacc(
        "TRN2", target_bir_lowering=False, debug=False, num_devices=NCORES
    )

    # ---- DRAM I/O (per-core shards, prearranged on host in SBUF layout) ----
    # xt:   [p, t, s]        = x^T[t*128+p, s]
    # wqk:  [p, kp, t*128+c] kp in (wq-p0, wk-p0, wq-p1, wk-p1), = W[t*128+p, pair-col c]
    # wv:   [p, t*256+c]     = W_V[t*128+p, c]   (c over all 4 heads)
    # wo:   [p, pair, c]     = W_O_cat[pair*128+p, c]
    # auxf: [p, 0:2]=bq pair cols, [2:4]=bk, [4:260]=bv row (bcast over p)
    # bandm:[p, o, c]        0/1 causal band masks
    WDT = FP8 if qk_fp8 else BF16
    xt_d = nc.dram_tensor("xt", [P, NDM, S], BF16, kind="ExternalInput")
    if qk_fp8:
        xt8_d = nc.dram_tensor("xt8", [P, NDM, S], FP8, kind="ExternalInput")
    wqk_d = nc.dram_tensor("wqk", [P, 4, NDM, P], WDT, kind="ExternalInput")
    wv_d = nc.dram_tensor("wv", [P, NDM * HPC * DH], BF16, kind="ExternalInput")
    wo_d = nc.dram_tensor("wo", [P, 2, DM], BF16, kind="ExternalInput")
    auxf_d = nc.dram_tensor("auxf", [P, 4 + HPC * DH], F32, kind="ExternalInput")
    bandm_d = nc.dram_tensor("bandm", [P, 2, 2 * QB], BF16, kind="ExternalInput")
    out_d = nc.dram_tensor("out", [S, DM], BF16, kind="ExternalOutput")

    with tile.TileContext(nc) as tc, ExitStack() as ctx:
        const = ctx.enter_context(tc.tile_pool(name="const", bufs=1))
        persist = ctx.enter_context(tc.tile_pool(name="persist", bufs=1))

        # ---- SBUF persistent tensors ----
        xt_sb = persist.tile([P, NDM, S], BF16, name="xt_sb", tag="xt")
        if qk_fp8:
            xt8_sb = persist.tile([P, NDM, S], FP8, name="xt8_sb", tag="xt8")
        wqk_sb = persist.tile([P, 4, NDM, P], WDT, name="wqk_sb", tag="wqk")
        wv_sb = persist.tile([P, NDM * HPC * DH], BF16, name="wv_sb", tag="wv")
        wo_sb = persist.tile([P, 2, DM], BF16, name="wo_sb", tag="wo")
        auxf_sb = persist.tile([P, 4 + HPC * DH], F32, name="auxf_sb", tag="auxf")
        bandm_sb = persist.tile([P, 2, 2 * QB], BF16, name="bandm_sb", tag="bandm")
        qt_sb = [
            persist.tile([P, S], BF16, name=f"qt{p}", tag=f"qt{p}") for p in range(2)
        ]
        kt_sb = [
            persist.tile([P, S], BF16, name=f"kt{p}", tag=f"kt{p}") for p in range(2)
        ]
        v_sb = [
            persist.tile([P, NKT, P], BF16, name=f"v{p}", tag=f"v{p}")
            for p in range(2)
        ]
        zt_sb = [
            persist.tile([P, S], BF16, name=f"zt{p}", tag=f"zt{p}") for p in range(2)
        ]
        ones64 = const.tile([P, 64], BF16, name="ones64", tag="ones64")
        warm_in = const.tile([P, QB], BF16, name="warm_in", tag="warm_in")
        warm_out = const.tile([P, 8], F32, name="warm_out", tag="warm_out")

        # ---- PSUM pools: 2*2 (scores) + 1 (z) + 1 (d) + 2 (proj/O) = 8 banks
        sp = ctx.enter_context(tc.tile_pool(name="sp", bufs=2, space="PSUM"))
        zp = ctx.enter_context(tc.tile_pool(name="zp", bufs=1, space="PSUM"))
        dp = ctx.enter_context(tc.tile_pool(name="dp", bufs=1, space="PSUM"))
        pj = ctx.enter_context(tc.tile_pool(name="pj", bufs=2, space="PSUM"))

        ppool = ctx.enter_context(tc.tile_pool(name="ppool", bufs=8))
        bcpool = ctx.enter_context(tc.tile_pool(name="bcpool", bufs=2))
        ost = ctx.enter_context(tc.tile_pool(name="ost", bufs=3))

        # ---- warmup: runs during the input DMA wait ----
        nc.gpsimd.memset(ones64[:], 1.0)
        nc.gpsimd.memset(warm_in[:], 0.0)
        # preload the exp table set (~2.7us) before the first real exp
        nc.scalar.activation(
            warm_out[:], warm_in[:, 0:8], mybir.ActivationFunctionType.Exp,
            scale=1.0,
        )
        for w in range(NWARM):
            wps = pj.tile([P, QB], F32, name="wps", tag="pj")
            _mm(nc, wps[0:64, :], ones64[:], warm_in[:], start=True, stop=True)

        # ---- input DMAs (sync queue), in compute-readiness order ----
        if qk_fp8:
            # Q/K path reads fp8 copies; bf16 x feeds only the V projection.
            # First x chunk split in half so the first matmuls start sooner.
            nc.sync.dma_start(out=wqk_sb[:], in_=wqk_d[:, :, :, :])
            nc.sync.dma_start(out=xt8_sb[:, 0:4, 0:QB], in_=xt8_d[:, 0:4, 0:QB])
            nc.sync.dma_start(out=xt8_sb[:, 4:8, 0:QB], in_=xt8_d[:, 4:8, 0:QB])
            nc.sync.dma_start(out=auxf_sb[:], in_=auxf_d[:, :])
            nc.sync.dma_start(out=xt_sb[:, :, 0:QB], in_=xt_d[:, :, 0:QB])
            nc.sync.dma_start(out=wv_sb[:], in_=wv_d[:, :])
            nc.sync.dma_start(out=bandm_sb[:], in_=bandm_d[:, :, :])
            for ch in range(1, NQB):
                c0, c1 = ch * QB, (ch + 1) * QB
                nc.sync.dma_start(out=xt8_sb[:, :, c0:c1], in_=xt8_d[:, :, c0:c1])
                nc.sync.dma_start(out=xt_sb[:, :, c0:c1], in_=xt_d[:, :, c0:c1])
            nc.sync.dma_start(out=wo_sb[:], in_=wo_d[:, :, :])
        else:
            nc.sync.dma_start(out=wqk_sb[:, 0], in_=wqk_d[:, 0])   # wq pair0
            nc.sync.dma_start(out=xt_sb[:, :, 0:QB], in_=xt_d[:, :, 0:QB])
            nc.sync.dma_start(out=auxf_sb[:], in_=auxf_d[:, :])
            nc.sync.dma_start(out=wqk_sb[:, 1], in_=wqk_d[:, 1])   # wk pair0
            nc.sync.dma_start(out=wv_sb[:], in_=wv_d[:, :])
            nc.sync.dma_start(out=bandm_sb[:], in_=bandm_d[:, :, :])
            nc.sync.dma_start(
                out=xt_sb[:, :, QB : 2 * QB], in_=xt_d[:, :, QB : 2 * QB]
            )
            nc.sync.dma_start(out=wqk_sb[:, 2], in_=wqk_d[:, 2])   # wq pair1
            nc.sync.dma_start(out=wqk_sb[:, 3], in_=wqk_d[:, 3])   # wk pair1
            nc.sync.dma_start(
                out=xt_sb[:, :, 2 * QB : 3 * QB], in_=xt_d[:, :, 2 * QB : 3 * QB]
            )
            nc.sync.dma_start(
                out=xt_sb[:, :, 3 * QB : 4 * QB], in_=xt_d[:, :, 3 * QB : 4 * QB]
            )
            nc.sync.dma_start(out=wo_sb[:], in_=wo_d[:, :, :])

        bq_sb = auxf_sb[:, 0:2]
        bk_sb = auxf_sb[:, 2:4]
        bv_sb = auxf_sb[:, 4 : 4 + HPC * DH]

        def qk_chunk(p, ch):
            # Q^T and K^T chunk ch for pair p: [dh-pair (128), 512 q]
            for dst, kp, bias in (
                (qt_sb, 2 * p, bq_sb),
                (kt_sb, 2 * p + 1, bk_sb),
            ):
                qp = pj.tile([P, QB], F32, name="qp", tag="pj")
                if qk_fp8:
                    # fp8 DoubleRow: 2 dm-tiles (planes) per pass
                    for t2 in range(NDM // 2):
                        nc.tensor.matmul(
                            qp[:],
                            wqk_sb[:, kp, 2 * t2 : 2 * t2 + 2, :],
                            xt8_sb[:, 2 * t2 : 2 * t2 + 2,
                                   ch * QB : (ch + 1) * QB],
                            start=(t2 == 0),
                            stop=(t2 == NDM // 2 - 1),
                            perf_mode=mybir.MatmulPerfMode.DoubleRow,
                        )
                else:
                    for t in range(NDM):
                        _mm(
                            nc,
                            qp[:],
                            wqk_sb[:, kp, t, :],
                            xt_sb[:, t, ch * QB : (ch + 1) * QB],
                            start=(t == 0),
                            stop=(t == NDM - 1),
                        )
                nc.vector.tensor_scalar_add(
                    dst[p][:, ch * QB : (ch + 1) * QB],
                    qp[:],
                    bias[:, p : p + 1],
                )

        def v_tile(st):
            # V: [seq-tile, 4 heads dh] -> per-pair tiles
            vp = pj.tile([P, QB], F32, name="vp", tag="pj")
            for t in range(NDM):
                _mm(
                    nc,
                    vp[:, 0 : HPC * DH],
                    xt_sb[:, t, st * P : (st + 1) * P],
                    wv_sb[:, t * HPC * DH : (t + 1) * HPC * DH],
                    start=(t == 0),
                    stop=(t == NDM - 1),
                )
            for p in range(2):
                nc.vector.tensor_add(
                    v_sb[p][:, st, :],
                    vp[:, p * P : (p + 1) * P],
                    bv_sb[:, p * P : (p + 1) * P],
                )

        # scores arrive scaled by WSCALE^2 in fp8 mode; fold into the exp scale
        EXP_SCALE = 0.125 / (WSCALE * WSCALE if qk_fp8 else 1.0)

        def attn_qb(p, qb):
            q0 = qb * QB
            nk = (qb + 1) * (QB // P)  # k tiles in causal range
            zps = zp.tile([P, QB], F32, name="zps", tag="z")
            dnb = dp.tile([P, QB], F32, name="dnb", tag="d")

            def pv_dnb(pA, pB, kg, pA01, pB01):
                # PV (column-packed heads) + softmax denominators: the
                # ones-matmul sums the gpsimd-presummed P planes over k AND
                # broadcasts over the 64 rows of each head half; one dnb
                # pass per k-group instead of one per k-tile.
                nkg = nk // 2
                c0g = max(kg * 2 * P - q0, 0)
                ins = []
                for j in range(2):
                    kt = kg * 2 + j
                    c0 = max(kt * P - q0, 0)
                    ins += [
                        _mm(
                            nc, zps[0:64, c0:QB], v_sb[p][:, kt, 0:64],
                            pA[:, j, c0:QB],
                            start=(kt == 0), stop=(kt == nk - 1), skip=True,
                        ),
                        _mm(
                            nc, zps[64:P, c0:QB], v_sb[p][:, kt, 64:P],
                            pB[:, j, c0:QB],
                            start=(kt == 0), stop=(kt == nk - 1), skip=True,
                        ),
                    ]
                ins += [
                    _mm(
                        nc, dnb[0:64, c0g:QB], ones64[:], pA01[:, c0g:QB],
                        start=(kg == 0), stop=(kg == nkg - 1), skip=True,
                    ),
                    _mm(
                        nc, dnb[64:P, c0g:QB], ones64[:], pB01[:, c0g:QB],
                        start=(kg == 0), stop=(kg == nkg - 1), skip=True,
                    ),
                ]
                _chain(ins)

            for kg in range(nk // 2):
                # offs[j]: first valid q column of k-tile kg*2+j
                offs = [kg * 2 * P + j * P - q0 for j in range(2)]
                band = offs[0] >= 0
                deep = band and offs[0] >= 2 * P  # o=1 band k-group
                sA = sp.tile([P, 2, QB], F32, name="sA", tag="s")
                sB = sp.tile([P, 2, QB], F32, name="sB", tag="s")
                for j in range(2):
                    # band k-tiles only compute scores for columns with any
                    # unmasked row; the skipped region holds stale (bounded)
                    # psum that exp+mask neutralizes. The very first k-group
                    # computes everything -- its psum slots are uninitialized.
                    first = qb == 0 and kg == 0
                    c0 = max(offs[j], 0) if band and not first else 0
                    _chain([
                        _mm(
                            nc,
                            stile[:, j, c0:QB],
                            kt_sb[p][rows, (kg * 2 + j) * P : (kg * 2 + j + 1) * P],
                            qt_sb[p][rows, q0 + c0 : q0 + QB],
                            start=True,
                            stop=True,
                        )
                        for rows, stile in ((slice(0, 64), sA), (slice(64, P), sB))
                    ])
                pA = ppool.tile([P, 2, QB], BF16, name="pA", tag="pt")
                pB = ppool.tile([P, 2, QB], BF16, name="pB", tag="pt")
                # exp(S/sqrt(dh)); scale folded into ACT
                if deep:
                    # left of offs[0] is fully masked for both j: zero it and
                    # exp the rest in one shot (j1's leading slice is stale
                    # but bounded; the band mask zeroes it below)
                    for px, sx in ((pA, sA), (pB, sB)):
                        nc.vector.memset(px[:, :, 0 : offs[0]], 0.0)
                        nc.scalar.activation(
                            px[:, :, offs[0] : QB],
                            sx[:, :, offs[0] : QB],
                            mybir.ActivationFunctionType.Exp,
                            scale=EXP_SCALE,
                        )
                else:
                    nc.scalar.activation(
                        pA[:], sA[:], mybir.ActivationFunctionType.Exp,
                        scale=EXP_SCALE,
                    )
                    nc.scalar.activation(
                        pB[:], sB[:], mybir.ActivationFunctionType.Exp,
                        scale=EXP_SCALE,
                    )
                if band:
                    # causal mask: multiply diagonal-band P tiles by 0/1
                    o = offs[0] // (2 * P)
                    nc.vector.tensor_mul(pA[:], pA[:], bandm_sb[:, o, :])
                    nc.vector.tensor_mul(pB[:], pB[:], bandm_sb[:, o, :])
                # presummed P planes (gpsimd, idle engine) halve the dnb
                # matmul passes
                c0g = max(offs[0], 0)
                pA01 = ppool.tile([P, QB], BF16, name="pA01", tag="ps")
                pB01 = ppool.tile([P, QB], BF16, name="pB01", tag="ps")
                nc.gpsimd.tensor_add(
                    pA01[:, c0g:QB], pA[:, 0, c0g:QB], pA[:, 1, c0g:QB]
                )
                nc.gpsimd.tensor_add(
                    pB01[:, c0g:QB], pB[:, 0, c0g:QB], pB[:, 1, c0g:QB]
                )
                pv_dnb(pA, pB, kg, pA01, pB01)

            bcs = bcpool.tile([P, QB], F32, name="bcs", tag="bcs")
            bcr = bcpool.tile([P, QB], F32, name="bcr", tag="bcr")
            nc.vector.reciprocal_approx_accurate(
                out=bcr[:], in_=dnb[:], scratch=bcs[:]
            )
            nc.vector.tensor_mul(zt_sb[p][:, q0 : q0 + QB], zps[:], bcr[:])

        def o_chunk(st, tail=False):
            # O partial rows st*128..: contraction over both pairs' Z^T.
            # In the tail (after the last exp) the scalar engine is idle, so
            # split the copy work across scalar+vector and DMA each half
            # as soon as it lands.
            ot = ost.tile([P, DM], BF16, name="ot", tag="ot")
            for nn in range(2):
                ops = pj.tile([P, QB], F32, name="ops", tag="pj")
                for pp in range(2):
                    _mm(
                        nc,
                        ops[:],
                        zt_sb[pp][:, st * P : (st + 1) * P],
                        wo_sb[:, pp, nn * QB : (nn + 1) * QB],
                        start=(pp == 0),
                        stop=(pp == 1),
                    )
                if tail and nn == 1:
                    nc.scalar.copy(ot[:, nn * QB : (nn + 1) * QB], ops[:])
                else:
                    nc.vector.tensor_copy(ot[:, nn * QB : (nn + 1) * QB], ops[:])
                if tail:
                    nc.sync.dma_start(
                        out=out_d[st * P : (st + 1) * P,
                                  nn * QB : (nn + 1) * QB],
                        in_=ot[:, nn * QB : (nn + 1) * QB],
                    )
            if not tail:
                nc.sync.dma_start(
                    out=out_d[st * P : (st + 1) * P, :], in_=ot[:]
                )

        # ---- pipelined emission ----
        for ch in range(NQB):
            qk_chunk(0, ch)
            for st in range(4 * ch, 4 * ch + 4):
                v_tile(st)
            attn_qb(0, ch)
        for ch in range(NQB):
            qk_chunk(1, ch)
            attn_qb(1, ch)
            if ch >= 1:
                for st in range(4 * (ch - 1), 4 * ch):
                    o_chunk(st)
        for st in range(12, 16):
            o_chunk(st, tail=True)

    nc.compile()
    _PROGRAM_CACHE["v2"] = nc
    return nc


def make_in_maps(
    normalized_resid_pre, W_Q, W_K, W_V, W_O, b_Q, b_K, b_V, b_O,
    qk_fp8=QK_FP8,
):
    """Shard + prearrange the full inputs into per-core input maps."""
    import ml_dtypes  # noqa: F401  (registers bfloat16 with numpy)

    bf16 = np.dtype("bfloat16")
    fp8 = np.dtype(ml_dtypes.float8_e4m3)  # TRN fp8e4 (max 240) variant

    x = np.asarray(normalized_resid_pre, dtype=np.float32)
    W_Q = np.asarray(W_Q, dtype=np.float32)
    W_K = np.asarray(W_K, dtype=np.float32)
    W_V = np.asarray(W_V, dtype=np.float32)
    W_O = np.asarray(W_O, dtype=np.float32)
    b_Q = np.asarray(b_Q, dtype=np.float32)
    b_K = np.asarray(b_K, dtype=np.float32)
    b_V = np.asarray(b_V, dtype=np.float32)

    # xt[p, t, s] = x[b][s, t*128+p]
    xts = []
    for b in range(B):
        xt = np.ascontiguousarray(
            x[b].T.reshape(NDM, P, S).transpose(1, 0, 2)
        ).astype(bf16)
        xts.append(xt)

    # additive causal band masks at k-group granularity: variant o covers the
    # two k-tiles at q-block offsets (2o*128, (2o+1)*128)
    kp = np.arange(P)[:, None]
    qc = np.arange(QB)[None, :]
    bandm = np.stack(
        [
            np.concatenate(
                [
                    np.where(qc < (2 * o + j) * P + kp,
                             np.float32(0.0), np.float32(1.0))
                    for j in range(2)
                ],
                axis=1,
            )
            for o in range(2)
        ],
        axis=1,
    ).astype(bf16)  # [P, 2, 2*QB]

    in_maps = []
    for c in range(NCORES):
        b = c // (NCORES // B)
        heads = [HPC * (c % (NCORES // B)) + i for i in range(HPC)]
        wq_cat = np.concatenate([W_Q[h] for h in heads], axis=1)  # [DM, 256]
        wk_cat = np.concatenate([W_K[h] for h in heads], axis=1)
        wv_cat = np.concatenate([W_V[h] for h in heads], axis=1)
        wo_cat = np.concatenate([W_O[h] for h in heads], axis=0)  # [256, DM]

        # wqk[p, kp, t, cc] = W[t*128+p, pair*128 + cc]
        def pack_w(wcat, pair):
            wp = wcat[:, pair * P : (pair + 1) * P]         # [DM, 128]
            return wp.reshape(NDM, P, P).transpose(1, 0, 2)  # [P, NDM, P]

        wqk = np.stack(
            [pack_w(wq_cat, 0), pack_w(wk_cat, 0),
             pack_w(wq_cat, 1), pack_w(wk_cat, 1)],
            axis=1,
        )  # [P, 4, NDM, P] fp32
        if qk_fp8:
            wqk = (wqk * np.float32(WSCALE)).astype(fp8)
        else:
            wqk = wqk.astype(bf16)

        wv = (
            wv_cat.reshape(NDM, P, HPC * DH)
            .transpose(1, 0, 2)
            .reshape(P, NDM * HPC * DH)
            .astype(bf16)
        )
        wo = (
            wo_cat.reshape(2, P, DM).transpose(1, 0, 2).astype(bf16)
        )  # [P, 2, DM]

        # in fp8 mode Q''/K'' carry a WSCALE factor, so biases scale too
        bsc = np.float32(WSCALE if qk_fp8 else 1.0)
        auxf = np.zeros((P, 4 + HPC * DH), dtype=np.float32)
        auxf[:, 0] = np.concatenate([b_Q[heads[0]], b_Q[heads[1]]]) * bsc
        auxf[:, 1] = np.concatenate([b_Q[heads[2]], b_Q[heads[3]]]) * bsc
        auxf[:, 2] = np.concatenate([b_K[heads[0]], b_K[heads[1]]]) * bsc
        auxf[:, 3] = np.concatenate([b_K[heads[2]], b_K[heads[3]]]) * bsc
        auxf[:, 4:] = np.concatenate([b_V[h] for h in heads])[None, :]

        im = {
            "xt": np.ascontiguousarray(xts[b].reshape(P, NDM, S)),
            "wqk": np.ascontiguousarray(wqk),
            "wv": wv,
            "wo": np.ascontiguousarray(wo),
            "auxf": auxf,
            "bandm": np.ascontiguousarray(bandm),
        }
        if qk_fp8:
            im["xt8"] = np.ascontiguousarray(xts[b].astype(np.float32)).astype(
                fp8
            ).reshape(P, NDM, S)
        in_maps.append(im)
    return in_maps


def kernel(normalized_resid_pre, W_Q, W_K, W_V, W_O, b_Q, b_K, b_V, b_O):
    global LAST_RESULTS
    nc = build_program()
    in_maps = make_in_maps(
        normalized_resid_pre, W_Q, W_K, W_V, W_O, b_Q, b_K, b_V, b_O
    )
    trace = os.environ.get("ATTN_TRACE", "0") == "1"
    res = run_bass_kernel_spmd(nc, in_maps, list(range(NCORES)), trace=trace)
    LAST_RESULTS = res

    b_O = np.asarray(b_O, dtype=np.float32)
    parts = [
        np.asarray(res.results[c]["out"], dtype=np.float64) for c in range(NCORES)
    ]
    npc = NCORES // B  # cores per batch
    out = np.stack(
        [sum(parts[b * npc : (b + 1) * npc]) + b_O for b in range(B)]
    )
    return out.astype(np.float32)


# revision 23
# speedup vs baseline: 1.0897x; 1.0897x over previous
"""Trainium2 Bass kernel for causal multi-head attention (dense transformer block).

Problem (hardcoded): x [2, 2048, 1024], 16 heads x 64 dh, causal attention,
fp32 I/O. Sharding: 8 cores = 2 batches x 4 head-groups. Each core computes 4
heads for one batch plus a partial output projection [2048, 1024] (bf16); the
host sums the 4 partials per batch and adds b_O.

Everything on-device is computed in "transposed" orientation so no transposes
are needed anywhere:
  x^T (host-pretransposed)  ->  Q^T, K^T [dh, s] and V [s, dh] via matmuls
  S^T[k, q] = K Q^T         ->  P^T = exp(S^T / 8) (causal-masked pre-exp)
  Z^T[dh, q] = V^T P^T      ->  normalized by column sums (ones-matmul)
  O[s, :]   = (Z^T)^T W_O   (Z^T is directly the lhsT of the O-projection)

Heads are processed in pairs: QK^T packs 2 heads in row-groups (0-63 / 64-127)
of the PE array, PV packs 2 heads in column-groups -- both run concurrently.

v2 schedule: the whole kernel is software-pipelined in emission order so the
scalar engine (softmax exp, the per-core floor at ~58us) overlaps all other
work:
  warmup (PE HAM + ACT table) during the input DMA wait
  for ch: qk_proj(pair0, ch) + v_proj(4 tiles); attention(pair0, qb=ch)
  for ch: qk_proj(pair1, ch); attention(pair1, qb=ch); O-proj chunks
PSUM: scores 2x2 banks, zps 1, dnb 1, proj/O 2 = 8 banks.
"""

import os
from contextlib import ExitStack

import numpy as np

import concourse.tile as tile
from concourse import bacc, mybir
from concourse.bass_utils import run_bass_kernel_spmd

# problem constants
B, S, DM, H, DH = 2, 2048, 1024, 16, 64
P = 128          # partitions
QB = 512         # q block (matmul moving free dim)
NKT = S // P     # 16 k tiles
NQB = S // QB    # 4 q blocks
NDM = DM // P    # 8 d_model tiles
HPC = 4          # heads per core
NCORES = 8
NWARM = 9        # PE warmup matmuls (~3.5us to beat the HAM cold clock)

F32 = mybir.dt.float32
BF16 = mybir.dt.bfloat16
FP8 = mybir.dt.float8e4

# fp8 DoubleRow Q/K projections: W_Q/W_K and x are quantized to fp8e4 on the
# host (weights pre-scaled by 64 so they clear the fp8 subnormal range; the
# 1/64^2 un-scale is folded into the softmax exp scale). Q/K quantization
# noise is random across the contraction and averages out through the
# softmax, unlike V/W_O noise which lands directly in the output.
QK_FP8 = os.environ.get("ATTN_QK_FP8", "1") == "1"
WSCALE = 64.0

_PROGRAM_CACHE = {}
LAST_RESULTS = None  # BassKernelResults of the most recent run (for test.py)


def _mm(nc, out, lhsT, rhs, start, stop, skip=False):
    # skip_group_check: the sim's psum-group tracker doesn't distinguish
    # partition ranges; our concurrent groups in one bank are partition-disjoint
    # (rows 0-63 vs 64-127), which the per-partition zeroing model handles.
    return nc.tensor.matmul(
        out, lhsT, rhs, start=start, stop=stop, skip_group_check=skip
    )


def _chain(insts):
    """Ordering-only PE edges so matmuls alternating between row/column
    groups stay adjacent and run concurrently on the array."""
    from concourse.tile import add_dep_helper

    for a, b in zip(insts[1:], insts):
        add_dep_helper(a.ins, b.ins, sync=False, reason="pack-pair order")


def build_program(qk_fp8=QK_FP8):
    """Build the single-core SPMD Bass program (same program on all 8 cores)."""
    if qk_fp8 in _PROGRAM_CACHE:
        return _PROGRAM_CACHE[qk_fp8]

    nc = bacc.Bacc(
        "TRN2", target_bir_lowering=False, debug=False, num_devices=NCORES
    )

    # ---- DRAM I/O (per-core shards, prearranged on host in SBUF layout) ----
    # xt:   [p, t, s]        = x^T[t*128+p, s]
    # wqk:  [p, kp, t*128+c] kp in (wq-p0, wk-p0, wq-p1, wk-p1), = W[t*128+p, pair-col c]
    # wv:   [p, t*256+c]     = W_V[t*128+p, c]   (c over all 4 heads)
    # wo:   [p, pair, c]     = W_O_cat[pair*128+p, c]
    # auxf: [p, 0:2]=bq pair cols, [2:4]=bk, [4:260]=bv row (bcast over p)
    # bandm:[p, o, c]        0/1 causal band masks
    WDT = FP8 if qk_fp8 else BF16
    xt_d = nc.dram_tensor("xt", [P, NDM, S], BF16, kind="ExternalInput")
    if qk_fp8:
        xt8_d = nc.dram_tensor("xt8", [P, NDM, S], FP8, kind="ExternalInput")
    wqk_d = nc.dram_tensor("wqk", [P, 4, NDM, P], WDT, kind="ExternalInput")
    wv_d = nc.dram_tensor("wv", [P, NDM * HPC * DH], BF16, kind="ExternalInput")
    wo_d = nc.dram_tensor("wo", [P, 2, DM], BF16, kind="ExternalInput")
    auxf_d = nc.dram_tensor("auxf", [P, 4 + HPC * DH], F32, kind="ExternalInput")
    bandm_d = nc.dram_tensor("bandm", [P, 2, 2 * QB], BF16, kind="ExternalInput")
    out_d = nc.dram_tensor("out", [S, DM], BF16, kind="ExternalOutput")

    with tile.TileContext(nc) as tc, ExitStack() as ctx:
        const = ctx.enter_context(tc.tile_pool(name="const", bufs=1))
        persist = ctx.enter_context(tc.tile_pool(name="persist", bufs=1))

        # ---- SBUF persistent tensors ----
        xt_sb = persist.tile([P, NDM, S], BF16, name="xt_sb", tag="xt")
        if qk_fp8:
            xt8_sb = persist.tile([P, NDM, S], FP8, name="xt8_sb", tag="xt8")
        wqk_sb = persist.tile([P, 4, NDM, P], WDT, name="wqk_sb", tag="wqk")
        wv_sb = persist.tile([P, NDM * HPC * DH], BF16, name="wv_sb", tag="wv")
        wo_sb = persist.tile([P, 2, DM], BF16, name="wo_sb", tag="wo")
        auxf_sb = persist.tile([P, 4 + HPC * DH], F32, name="auxf_sb", tag="auxf")
        bandm_sb = persist.tile([P, 2, 2 * QB], BF16, name="bandm_sb", tag="bandm")
        qt_sb = [
            persist.tile([P, S], BF16, name=f"qt{p}", tag=f"qt{p}") for p in range(2)
        ]
        kt_sb = [
            persist.tile([P, S], BF16, name=f"kt{p}", tag=f"kt{p}") for p in range(2)
        ]
        v_sb = [
            persist.tile([P, NKT, P], BF16, name=f"v{p}", tag=f"v{p}")
            for p in range(2)
        ]
        zt_sb = [
            persist.tile([P, S], BF16, name=f"zt{p}", tag=f"zt{p}") for p in range(2)
        ]
        ones64 = const.tile([P, 64], BF16, name="ones64", tag="ones64")
        warm_in = const.tile([P, QB], BF16, name="warm_in", tag="warm_in")
        warm_out = const.tile([P, 8], F32, name="warm_out", tag="warm_out")

        # ---- PSUM pools: 2*2 (scores) + 1 (z) + 1 (d) + 2 (proj/O) = 8 banks
        sp = ctx.enter_context(tc.tile_pool(name="sp", bufs=2, space="PSUM"))
        zp = ctx.enter_context(tc.tile_pool(name="zp", bufs=1, space="PSUM"))
        dp = ctx.enter_context(tc.tile_pool(name="dp", bufs=1, space="PSUM"))
        pj = ctx.enter_context(tc.tile_pool(name="pj", bufs=2, space="PSUM"))

        ppool = ctx.enter_context(tc.tile_pool(name="ppool", bufs=8))
        bcpool = ctx.enter_context(tc.tile_pool(name="bcpool", bufs=2))
        ost = ctx.enter_context(tc.tile_pool(name="ost", bufs=3))

        # ---- warmup: runs during the input DMA wait ----
        nc.gpsimd.memset(ones64[:], 1.0)
        nc.gpsimd.memset(warm_in[:], 0.0)
        # preload the exp table set (~2.7us) before the first real exp
        nc.scalar.activation(
            warm_out[:], warm_in[:, 0:8], mybir.ActivationFunctionType.Exp,
            scale=1.0,
        )
        for w in range(NWARM):
            wps = pj.tile([P, QB], F32, name="wps", tag="pj")
            _mm(nc, wps[0:64, :], ones64[:], warm_in[:], start=True, stop=True)

        # ---- input DMAs (sync queue), in compute-readiness order ----
        if qk_fp8:
            # Q/K path reads fp8 copies; bf16 x feeds only the V projection.
            # First x chunk split in half so the first matmuls start sooner.
            nc.sync.dma_start(out=wqk_sb[:], in_=wqk_d[:, :, :, :])
            nc.sync.dma_start(out=xt8_sb[:, 0:4, 0:QB], in_=xt8_d[:, 0:4, 0:QB])
            nc.sync.dma_start(out=xt8_sb[:, 4:8, 0:QB], in_=xt8_d[:, 4:8, 0:QB])
            nc.sync.dma_start(out=auxf_sb[:], in_=auxf_d[:, :])
            nc.sync.dma_start(out=xt_sb[:, :, 0:QB], in_=xt_d[:, :, 0:QB])
            nc.sync.dma_start(out=wv_sb[:], in_=wv_d[:, :])
            nc.sync.dma_start(out=bandm_sb[:], in_=bandm_d[:, :, :])
            for ch in range(1, NQB):
                c0, c1 = ch * QB, (ch + 1) * QB
                nc.sync.dma_start(out=xt8_sb[:, :, c0:c1], in_=xt8_d[:, :, c0:c1])
                nc.sync.dma_start(out=xt_sb[:, :, c0:c1], in_=xt_d[:, :, c0:c1])
            nc.sync.dma_start(out=wo_sb[:], in_=wo_d[:, :, :])
        else:
            nc.sync.dma_start(out=wqk_sb[:, 0], in_=wqk_d[:, 0])   # wq pair0
            nc.sync.dma_start(out=xt_sb[:, :, 0:QB], in_=xt_d[:, :, 0:QB])
            nc.sync.dma_start(out=auxf_sb[:], in_=auxf_d[:, :])
            nc.sync.dma_start(out=wqk_sb[:, 1], in_=wqk_d[:, 1])   # wk pair0
            nc.sync.dma_start(out=wv_sb[:], in_=wv_d[:, :])
            nc.sync.dma_start(out=bandm_sb[:], in_=bandm_d[:, :, :])
            nc.sync.dma_start(
                out=xt_sb[:, :, QB : 2 * QB], in_=xt_d[:, :, QB : 2 * QB]
            )
            nc.sync.dma_start(out=wqk_sb[:, 2], in_=wqk_d[:, 2])   # wq pair1
            nc.sync.dma_start(out=wqk_sb[:, 3], in_=wqk_d[:, 3])   # wk pair1
            nc.sync.dma_start(
                out=xt_sb[:, :, 2 * QB : 3 * QB], in_=xt_d[:, :, 2 * QB : 3 * QB]
            )
            nc.sync.dma_start(
                out=xt_sb[:, :, 3 * QB : 4 * QB], in_=xt_d[:, :, 3 * QB : 4 * QB]
            )
            nc.sync.dma_start(out=wo_sb[:], in_=wo_d[:, :, :])

        bq_sb = auxf_sb[:, 0:2]
        bk_sb = auxf_sb[:, 2:4]
        bv_sb = auxf_sb[:, 4 : 4 + HPC * DH]

        def qk_chunk(p, ch):
            # Q^T and K^T chunk ch for pair p: [dh-pair (128), 512 q]
            for dst, kp, bias in (
                (qt_sb, 2 * p, bq_sb),
                (kt_sb, 2 * p + 1, bk_sb),
            ):
                qp = pj.tile([P, QB], F32, name="qp", tag="pj")
                if qk_fp8:
                    # fp8 DoubleRow: 2 dm-tiles (planes) per pass
                    for t2 in range(NDM // 2):
                        nc.tensor.matmul(
                            qp[:],
                            wqk_sb[:, kp, 2 * t2 : 2 * t2 + 2, :],
                            xt8_sb[:, 2 * t2 : 2 * t2 + 2,
                                   ch * QB : (ch + 1) * QB],
                            start=(t2 == 0),
                            stop=(t2 == NDM // 2 - 1),
                            perf_mode=mybir.MatmulPerfMode.DoubleRow,
                        )
                else:
                    for t in range(NDM):
                        _mm(
                            nc,
                            qp[:],
                            wqk_sb[:, kp, t, :],
                            xt_sb[:, t, ch * QB : (ch + 1) * QB],
                            start=(t == 0),
                            stop=(t == NDM - 1),
                        )
                nc.vector.tensor_scalar_add(
                    dst[p][:, ch * QB : (ch + 1) * QB],
                    qp[:],
                    bias[:, p : p + 1],
                )

        def v_tile(st):
            # V: [seq-tile, 4 heads dh] -> per-pair tiles
            vp = pj.tile([P, QB], F32, name="vp", tag="pj")
            for t in range(NDM):
                _mm(
                    nc,
                    vp[:, 0 : HPC * DH],
                    xt_sb[:, t, st * P : (st + 1) * P],
                    wv_sb[:, t * HPC * DH : (t + 1) * HPC * DH],
                    start=(t == 0),
                    stop=(t == NDM - 1),
                )
            for p in range(2):
                nc.vector.tensor_add(
                    v_sb[p][:, st, :],
                    vp[:, p * P : (p + 1) * P],
                    bv_sb[:, p * P : (p + 1) * P],
                )

        # scores arrive scaled by WSCALE^2 in fp8 mode; fold into the exp scale
        EXP_SCALE = 0.125 / (WSCALE * WSCALE if qk_fp8 else 1.0)

        def attn_qb(p, qb):
            q0 = qb * QB
            nk = (qb + 1) * (QB // P)  # k tiles in causal range
            zps = zp.tile([P, QB], F32, name="zps", tag="z")
            dnb = dp.tile([P, QB], F32, name="dnb", tag="d")

            def pv_dnb(pA, pB, kg, pA01, pB01):
                # PV (column-packed heads) + softmax denominators: the
                # ones-matmul sums the gpsimd-presummed P planes over k AND
                # broadcasts over the 64 rows of each head half; one dnb
                # pass per k-group instead of one per k-tile.
                nkg = nk // 2
                c0g = max(kg * 2 * P - q0, 0)
                ins = []
                for j in range(2):
                    kt = kg * 2 + j
                    c0 = max(kt * P - q0, 0)
                    ins += [
                        _mm(
                            nc, zps[0:64, c0:QB], v_sb[p][:, kt, 0:64],
                            pA[:, j, c0:QB],
                            start=(kt == 0), stop=(kt == nk - 1), skip=True,
                        ),
                        _mm(
                            nc, zps[64:P, c0:QB], v_sb[p][:, kt, 64:P],
                            pB[:, j, c0:QB],
                            start=(kt == 0), stop=(kt == nk - 1), skip=True,
                        ),
                    ]
                ins += [
                    _mm(
                        nc, dnb[0:64, c0g:QB], ones64[:], pA01[:, c0g:QB],
                        start=(kg == 0), stop=(kg == nkg - 1), skip=True,
                    ),
                    _mm(
                        nc, dnb[64:P, c0g:QB], ones64[:], pB01[:, c0g:QB],
                        start=(kg == 0), stop=(kg == nkg - 1), skip=True,
                    ),
                ]
                _chain(ins)

            for kg in range(nk // 2):
                # offs[j]: first valid q column of k-tile kg*2+j
                offs = [kg * 2 * P + j * P - q0 for j in range(2)]
                band = offs[0] >= 0
                deep = band and offs[0] >= 2 * P  # o=1 band k-group
                sA = sp.tile([P, 2, QB], F32, name="sA", tag="s")
                sB = sp.tile([P, 2, QB], F32, name="sB", tag="s")
                for j in range(2):
                    # band k-tiles only compute scores for columns with any
                    # unmasked row; the skipped region holds stale (bounded)
                    # psum that exp+mask neutralizes. The very first k-group
                    # computes everything -- its psum slots are uninitialized.
                    first = qb == 0 and kg == 0
                    c0 = max(offs[j], 0) if band and not first else 0
                    _chain([
                        _mm(
                            nc,
                            stile[:, j, c0:QB],
                            kt_sb[p][rows, (kg * 2 + j) * P : (kg * 2 + j + 1) * P],
                            qt_sb[p][rows, q0 + c0 : q0 + QB],
                            start=True,
                            stop=True,
                        )
                        for rows, stile in ((slice(0, 64), sA), (slice(64, P), sB))
                    ])
                pA = ppool.tile([P, 2, QB], BF16, name="pA", tag="pt")
                pB = ppool.tile([P, 2, QB], BF16, name="pB", tag="pt")
                # exp(S/sqrt(dh)); scale folded into ACT
                if deep:
                    # left of offs[0] is fully masked for both j: zero it and
                    # exp the rest in one shot (j1's leading slice is stale
                    # but bounded; the band mask zeroes it below)
                    for px, sx in ((pA, sA), (pB, sB)):
                        nc.vector.memset(px[:, :, 0 : offs[0]], 0.0)
                        nc.scalar.activation(
                            px[:, :, offs[0] : QB],
                            sx[:, :, offs[0] : QB],
                            mybir.ActivationFunctionType.Exp,
                            scale=EXP_SCALE,
                        )
                else:
                    nc.scalar.activation(
                        pA[:], sA[:], mybir.ActivationFunctionType.Exp,
                        scale=EXP_SCALE,
                    )
                    nc.scalar.activation(
                        pB[:], sB[:], mybir.ActivationFunctionType.Exp,
                        scale=EXP_SCALE,
                    )
                if band:
                    # causal mask: multiply diagonal-band P tiles by 0/1
                    o = offs[0] // (2 * P)
                    nc.vector.tensor_mul(pA[:], pA[:], bandm_sb[:, o, :])
                    nc.vector.tensor_mul(pB[:], pB[:], bandm_sb[:, o, :])
                # presummed P planes (gpsimd, idle engine) halve the dnb
                # matmul passes
                c0g = max(offs[0], 0)
                pA01 = ppool.tile([P, QB], BF16, name="pA01", tag="ps")
                pB01 = ppool.tile([P, QB], BF16, name="pB01", tag="ps")
                nc.vector.tensor_add(
                    pA01[:, c0g:QB], pA[:, 0, c0g:QB], pA[:, 1, c0g:QB]
                )
                nc.vector.tensor_add(
                    pB01[:, c0g:QB], pB[:, 0, c0g:QB], pB[:, 1, c0g:QB]
                )
                pv_dnb(pA, pB, kg, pA01, pB01)

            bcs = bcpool.tile([P, QB], F32, name="bcs", tag="bcs")
            bcr = bcpool.tile([P, QB], F32, name="bcr", tag="bcr")
            nc.vector.reciprocal_approx_accurate(
                out=bcr[:], in_=dnb[:], scratch=bcs[:]
            )
            nc.vector.tensor_mul(zt_sb[p][:, q0 : q0 + QB], zps[:], bcr[:])

        def o_chunk(st, tail=False):
            # O partial rows st*128..: contraction over both pairs' Z^T.
            # In the tail (after the last exp) the scalar engine is idle, so
            # split the copy work across scalar+vector and DMA each half
            # as soon as it lands.
            ot = ost.tile([P, DM], BF16, name="ot", tag="ot")
            for nn in range(2):
                ops = pj.tile([P, QB], F32, name="ops", tag="pj")
                for pp in range(2):
                    _mm(
                        nc,
                        ops[:],
                        zt_sb[pp][:, st * P : (st + 1) * P],
                        wo_sb[:, pp, nn * QB : (nn + 1) * QB],
                        start=(pp == 0),
                        stop=(pp == 1),
                    )
                if tail and nn == 1:
                    nc.scalar.copy(ot[:, nn * QB : (nn + 1) * QB], ops[:])
                else:
                    nc.vector.tensor_copy(ot[:, nn * QB : (nn + 1) * QB], ops[:])
                if tail:
                    nc.sync.dma_start(
                        out=out_d[st * P : (st + 1) * P,
                                  nn * QB : (nn + 1) * QB],
                        in_=ot[:, nn * QB : (nn + 1) * QB],
                    )
            if not tail:
                nc.sync.dma_start(
                    out=out_d[st * P : (st + 1) * P, :], in_=ot[:]
                )

        # ---- pipelined emission ----
        # Phase A: pair-0 projections stream in with the input DMAs;
        # attention(0, qb) follows its chunk.
        for ch in range(NQB):
            qk_chunk(0, ch)
            for st in range(4 * ch, 4 * ch + 4):
                v_tile(st)
            attn_qb(0, ch)
        # Phase B: pair-1 projections fill the PE while attention(0)'s last
        # exp blocks drain on the scalar engine.
        for ch in range(NQB):
            qk_chunk(1, ch)
        # Phase C: attention(1) big-to-small so the largest exp block (qb3)
        # overlaps O-projection chunks and the smallest (qb0) is the tail.
        attn_qb(1, 3)
        for qb in (2, 1, 0):
            attn_qb(1, qb)
            for st in range(4 * (qb + 1), 4 * (qb + 2)):
                o_chunk(st)
        for st in range(4):
            o_chunk(st, tail=True)

    nc.compile()
    _PROGRAM_CACHE["v2"] = nc
    return nc


def make_in_maps(
    normalized_resid_pre, W_Q, W_K, W_V, W_O, b_Q, b_K, b_V, b_O,
    qk_fp8=QK_FP8,
):
    """Shard + prearrange the full inputs into per-core input maps."""
    import ml_dtypes  # noqa: F401  (registers bfloat16 with numpy)

    bf16 = np.dtype("bfloat16")
    fp8 = np.dtype(ml_dtypes.float8_e4m3)  # TRN fp8e4 (max 240) variant

    x = np.asarray(normalized_resid_pre, dtype=np.float32)
    W_Q = np.asarray(W_Q, dtype=np.float32)
    W_K = np.asarray(W_K, dtype=np.float32)
    W_V = np.asarray(W_V, dtype=np.float32)
    W_O = np.asarray(W_O, dtype=np.float32)
    b_Q = np.asarray(b_Q, dtype=np.float32)
    b_K = np.asarray(b_K, dtype=np.float32)
    b_V = np.asarray(b_V, dtype=np.float32)

    # xt[p, t, s] = x[b][s, t*128+p]
    xts = []
    for b in range(B):
        xt = np.ascontiguousarray(
            x[b].T.reshape(NDM, P, S).transpose(1, 0, 2)
        ).astype(bf16)
        xts.append(xt)

    # additive causal band masks at k-group granularity: variant o covers the
    # two k-tiles at q-block offsets (2o*128, (2o+1)*128)
    kp = np.arange(P)[:, None]
    qc = np.arange(QB)[None, :]
    bandm = np.stack(
        [
            np.concatenate(
                [
                    np.where(qc < (2 * o + j) * P + kp,
                             np.float32(0.0), np.float32(1.0))
                    for j in range(2)
                ],
                axis=1,
            )
            for o in range(2)
        ],
        axis=1,
    ).astype(bf16)  # [P, 2, 2*QB]

    in_maps = []
    for c in range(NCORES):
        b = c // (NCORES // B)
        heads = [HPC * (c % (NCORES // B)) + i for i in range(HPC)]
        wq_cat = np.concatenate([W_Q[h] for h in heads], axis=1)  # [DM, 256]
        wk_cat = np.concatenate([W_K[h] for h in heads], axis=1)
        wv_cat = np.concatenate([W_V[h] for h in heads], axis=1)
        wo_cat = np.concatenate([W_O[h] for h in heads], axis=0)  # [256, DM]

        # wqk[p, kp, t, cc] = W[t*128+p, pair*128 + cc]
        def pack_w(wcat, pair):
            wp = wcat[:, pair * P : (pair + 1) * P]         # [DM, 128]
            return wp.reshape(NDM, P, P).transpose(1, 0, 2)  # [P, NDM, P]

        wqk = np.stack(
            [pack_w(wq_cat, 0), pack_w(wk_cat, 0),
             pack_w(wq_cat, 1), pack_w(wk_cat, 1)],
            axis=1,
        )  # [P, 4, NDM, P] fp32
        if qk_fp8:
            wqk = (wqk * np.float32(WSCALE)).astype(fp8)
        else:
            wqk = wqk.astype(bf16)

        wv = (
            wv_cat.reshape(NDM, P, HPC * DH)
            .transpose(1, 0, 2)
            .reshape(P, NDM * HPC * DH)
            .astype(bf16)
        )
        wo = (
            wo_cat.reshape(2, P, DM).transpose(1, 0, 2).astype(bf16)
        )  # [P, 2, DM]

        # in fp8 mode Q''/K'' carry a WSCALE factor, so biases scale too
        bsc = np.float32(WSCALE if qk_fp8 else 1.0)
        auxf = np.zeros((P, 4 + HPC * DH), dtype=np.float32)
        auxf[:, 0] = np.concatenate([b_Q[heads[0]], b_Q[heads[1]]]) * bsc
        auxf[:, 1] = np.concatenate([b_Q[heads[2]], b_Q[heads[3]]]) * bsc
        auxf[:, 2] = np.concatenate([b_K[heads[0]], b_K[heads[1]]]) * bsc
        auxf[:, 3] = np.concatenate([b_K[heads[2]], b_K[heads[3]]]) * bsc
        auxf[:, 4:] = np.concatenate([b_V[h] for h in heads])[None, :]

        im = {
            "xt": np.ascontiguousarray(xts[b].reshape(P, NDM, S)),
            "wqk": np.ascontiguousarray(wqk),
            "wv": wv,
            "wo": np.ascontiguousarray(wo),
            "auxf": auxf,
            "bandm": np.ascontiguousarray(bandm),
        }
        if qk_fp8:
            im["xt8"] = np.ascontiguousarray(xts[b].astype(np.float32)).astype(
                fp8
            ).reshape(P, NDM, S)
        in_maps.append(im)
    return in_maps


def kernel(normalized_resid_pre, W_Q, W_K, W_V, W_O, b_Q, b_K, b_V, b_O):
    global LAST_RESULTS
    nc = build_program()
    in_maps = make_in_maps(
        normalized_resid_pre, W_Q, W_K, W_V, W_O, b_Q, b_K, b_V, b_O
    )
    trace = os.environ.get("ATTN_TRACE", "0") == "1"
    res = run_bass_kernel_spmd(nc, in_maps, list(range(NCORES)), trace=trace)
    LAST_RESULTS = res

    b_O = np.asarray(b_O, dtype=np.float32)
    parts = [
        np.asarray(res.results[c]["out"], dtype=np.float64) for c in range(NCORES)
    ]
    npc = NCORES // B  # cores per batch
    out = np.stack(
        [sum(parts[b * npc : (b + 1) * npc]) + b_O for b in range(B)]
    )
    return out.astype(np.float32)


# revision 26
# speedup vs baseline: 1.0980x; 1.0076x over previous
"""Trainium2 Bass kernel for causal multi-head attention (dense transformer block).

Problem (hardcoded): x [2, 2048, 1024], 16 heads x 64 dh, causal attention,
fp32 I/O. Sharding: 8 cores = 2 batches x 4 head-groups. Each core computes 4
heads for one batch plus a partial output projection [2048, 1024] (bf16); the
host sums the 4 partials per batch and adds b_O.

Everything on-device is computed in "transposed" orientation so no transposes
are needed anywhere:
  x^T (host-pretransposed)  ->  Q^T, K^T [dh, s] and V [s, dh] via matmuls
  S^T[k, q] = K Q^T         ->  P^T = exp(S^T / 8) (causal-masked pre-exp)
  Z^T[dh, q] = V^T P^T      ->  normalized by column sums (ones-matmul)
  O[s, :]   = (Z^T)^T W_O   (Z^T is directly the lhsT of the O-projection)

Heads are processed in pairs: QK^T packs 2 heads in row-groups (0-63 / 64-127)
of the PE array, PV packs 2 heads in column-groups -- both run concurrently.

v2 schedule: the whole kernel is software-pipelined in emission order so the
scalar engine (softmax exp, the per-core floor at ~58us) overlaps all other
work:
  warmup (PE HAM + ACT table) during the input DMA wait
  for ch: qk_proj(pair0, ch) + v_proj(4 tiles); attention(pair0, qb=ch)
  for ch: qk_proj(pair1, ch); attention(pair1, qb=ch); O-proj chunks
PSUM: scores 2x2 banks, zps 1, dnb 1, proj/O 2 = 8 banks.
"""

import os
from contextlib import ExitStack

import numpy as np

import concourse.tile as tile
from concourse import bacc, mybir
from concourse.bass_utils import run_bass_kernel_spmd

# problem constants
B, S, DM, H, DH = 2, 2048, 1024, 16, 64
P = 128          # partitions
QB = 512         # q block (matmul moving free dim)
NKT = S // P     # 16 k tiles
NQB = S // QB    # 4 q blocks
NDM = DM // P    # 8 d_model tiles
HPC = 4          # heads per core
NCORES = 8
NWARM = 12       # PE warmup matmuls; cover the HAM cold window + DMA wait

F32 = mybir.dt.float32
BF16 = mybir.dt.bfloat16
FP8 = mybir.dt.float8e4

# fp8 DoubleRow Q/K projections: W_Q/W_K and x are quantized to fp8e4 on the
# host (weights pre-scaled by 64 so they clear the fp8 subnormal range; the
# 1/64^2 un-scale is folded into the softmax exp scale). Q/K quantization
# noise is random across the contraction and averages out through the
# softmax, unlike V/W_O noise which lands directly in the output.
QK_FP8 = os.environ.get("ATTN_QK_FP8", "1") == "1"
WSCALE = 64.0

_PROGRAM_CACHE = {}
LAST_RESULTS = None  # BassKernelResults of the most recent run (for test.py)


def _mm(nc, out, lhsT, rhs, start, stop, skip=False):
    # skip_group_check: the sim's psum-group tracker doesn't distinguish
    # partition ranges; our concurrent groups in one bank are partition-disjoint
    # (rows 0-63 vs 64-127), which the per-partition zeroing model handles.
    return nc.tensor.matmul(
        out, lhsT, rhs, start=start, stop=stop, skip_group_check=skip
    )


def _chain(insts):
    """Ordering-only PE edges so matmuls alternating between row/column
    groups stay adjacent and run concurrently on the array."""
    from concourse.tile import add_dep_helper

    for a, b in zip(insts[1:], insts):
        add_dep_helper(a.ins, b.ins, sync=False, reason="pack-pair order")


def build_program(qk_fp8=QK_FP8):
    """Build the single-core SPMD Bass program (same program on all 8 cores)."""
    if qk_fp8 in _PROGRAM_CACHE:
        return _PROGRAM_CACHE[qk_fp8]

    nc = bacc.Bacc(
        "TRN2", target_bir_lowering=False, debug=False, num_devices=NCORES
    )

    # ---- DRAM I/O (per-core shards, prearranged on host in SBUF layout) ----
    # xt:   [p, t, s]        = x^T[t*128+p, s]
    # wqk:  [p, kp, t*128+c] kp in (wq-p0, wk-p0, wq-p1, wk-p1), = W[t*128+p, pair-col c]
    # wv:   [p, t*256+c]     = W_V[t*128+p, c]   (c over all 4 heads)
    # wo:   [p, pair, c]     = W_O_cat[pair*128+p, c]
    # auxf: [p, 0:2]=bq pair cols, [2:4]=bk, [4:260]=bv row (bcast over p)
    # bandm:[p, o, c]        0/1 causal band masks
    WDT = FP8 if qk_fp8 else BF16
    xt_d = nc.dram_tensor("xt", [P, NDM, S], BF16, kind="ExternalInput")
    if qk_fp8:
        xt8_d = nc.dram_tensor("xt8", [P, NDM, S], FP8, kind="ExternalInput")
    wqk_d = nc.dram_tensor("wqk", [P, 4, NDM, P], WDT, kind="ExternalInput")
    wv_d = nc.dram_tensor("wv", [P, NDM * HPC * DH], BF16, kind="ExternalInput")
    wo_d = nc.dram_tensor("wo", [P, 2, DM], BF16, kind="ExternalInput")
    auxf_d = nc.dram_tensor("auxf", [P, 4 + HPC * DH], F32, kind="ExternalInput")
    bandm_d = nc.dram_tensor("bandm", [P, 2, 2 * QB], BF16, kind="ExternalInput")
    out_d = nc.dram_tensor("out", [S, DM], BF16, kind="ExternalOutput")

    with tile.TileContext(nc) as tc, ExitStack() as ctx:
        const = ctx.enter_context(tc.tile_pool(name="const", bufs=1))
        persist = ctx.enter_context(tc.tile_pool(name="persist", bufs=1))

        # ---- SBUF persistent tensors ----
        xt_sb = persist.tile([P, NDM, S], BF16, name="xt_sb", tag="xt")
        if qk_fp8:
            xt8_sb = persist.tile([P, NDM, S], FP8, name="xt8_sb", tag="xt8")
        wqk_sb = persist.tile([P, 4, NDM, P], WDT, name="wqk_sb", tag="wqk")
        wv_sb = persist.tile([P, NDM * HPC * DH], BF16, name="wv_sb", tag="wv")
        wo_sb = persist.tile([P, 2, DM], BF16, name="wo_sb", tag="wo")
        auxf_sb = persist.tile([P, 4 + HPC * DH], F32, name="auxf_sb", tag="auxf")
        bandm_sb = persist.tile([P, 2, 2 * QB], BF16, name="bandm_sb", tag="bandm")
        qt_sb = [
            persist.tile([P, S], BF16, name=f"qt{p}", tag=f"qt{p}") for p in range(2)
        ]
        kt_sb = [
            persist.tile([P, S], BF16, name=f"kt{p}", tag=f"kt{p}") for p in range(2)
        ]
        v_sb = [
            persist.tile([P, NKT, P], BF16, name=f"v{p}", tag=f"v{p}")
            for p in range(2)
        ]
        zt_sb = [
            persist.tile([P, S], BF16, name=f"zt{p}", tag=f"zt{p}") for p in range(2)
        ]
        ones64 = const.tile([P, 64], BF16, name="ones64", tag="ones64")
        warm_in = const.tile([P, QB], BF16, name="warm_in", tag="warm_in")
        warm_out = const.tile([P, 8], F32, name="warm_out", tag="warm_out")

        # ---- PSUM pools: 2*2 (scores) + 1 (z) + 1 (d) + 2 (proj/O) = 8 banks
        sp = ctx.enter_context(tc.tile_pool(name="sp", bufs=2, space="PSUM"))
        zp = ctx.enter_context(tc.tile_pool(name="zp", bufs=1, space="PSUM"))
        dp = ctx.enter_context(tc.tile_pool(name="dp", bufs=1, space="PSUM"))
        pj = ctx.enter_context(tc.tile_pool(name="pj", bufs=2, space="PSUM"))

        ppool = ctx.enter_context(tc.tile_pool(name="ppool", bufs=8))
        bcpool = ctx.enter_context(tc.tile_pool(name="bcpool", bufs=2))
        ost = ctx.enter_context(tc.tile_pool(name="ost", bufs=3))

        # ---- warmup: runs during the input DMA wait ----
        nc.gpsimd.memset(ones64[:], 1.0)
        nc.gpsimd.memset(warm_in[:], 0.0)
        # preload the exp table set (~2.7us) before the first real exp
        nc.scalar.activation(
            warm_out[:], warm_in[:, 0:8], mybir.ActivationFunctionType.Exp,
            scale=1.0,
        )
        for w in range(NWARM):
            wps = pj.tile([P, QB], F32, name="wps", tag="pj")
            _mm(nc, wps[0:64, :], ones64[:], warm_in[:], start=True, stop=True)

        # ---- input DMAs (sync queue), in compute-readiness order ----
        if qk_fp8:
            # Q/K path reads fp8 copies; bf16 x feeds only the V projection.
            # First x chunk split in half so the first matmuls start sooner.
            nc.sync.dma_start(out=wqk_sb[:], in_=wqk_d[:, :, :, :])
            nc.sync.dma_start(out=xt8_sb[:, 0:4, 0:QB], in_=xt8_d[:, 0:4, 0:QB])
            nc.sync.dma_start(out=xt8_sb[:, 4:8, 0:QB], in_=xt8_d[:, 4:8, 0:QB])
            nc.sync.dma_start(out=auxf_sb[:], in_=auxf_d[:, :])
            nc.sync.dma_start(out=xt_sb[:, :, 0:QB], in_=xt_d[:, :, 0:QB])
            nc.sync.dma_start(out=wv_sb[:], in_=wv_d[:, :])
            nc.sync.dma_start(out=bandm_sb[:], in_=bandm_d[:, :, :])
            for ch in range(1, NQB):
                c0, c1 = ch * QB, (ch + 1) * QB
                nc.sync.dma_start(out=xt8_sb[:, :, c0:c1], in_=xt8_d[:, :, c0:c1])
                nc.sync.dma_start(out=xt_sb[:, :, c0:c1], in_=xt_d[:, :, c0:c1])
            nc.sync.dma_start(out=wo_sb[:], in_=wo_d[:, :, :])
        else:
            nc.sync.dma_start(out=wqk_sb[:, 0], in_=wqk_d[:, 0])   # wq pair0
            nc.sync.dma_start(out=xt_sb[:, :, 0:QB], in_=xt_d[:, :, 0:QB])
            nc.sync.dma_start(out=auxf_sb[:], in_=auxf_d[:, :])
            nc.sync.dma_start(out=wqk_sb[:, 1], in_=wqk_d[:, 1])   # wk pair0
            nc.sync.dma_start(out=wv_sb[:], in_=wv_d[:, :])
            nc.sync.dma_start(out=bandm_sb[:], in_=bandm_d[:, :, :])
            nc.sync.dma_start(
                out=xt_sb[:, :, QB : 2 * QB], in_=xt_d[:, :, QB : 2 * QB]
            )
            nc.sync.dma_start(out=wqk_sb[:, 2], in_=wqk_d[:, 2])   # wq pair1
            nc.sync.dma_start(out=wqk_sb[:, 3], in_=wqk_d[:, 3])   # wk pair1
            nc.sync.dma_start(
                out=xt_sb[:, :, 2 * QB : 3 * QB], in_=xt_d[:, :, 2 * QB : 3 * QB]
            )
            nc.sync.dma_start(
                out=xt_sb[:, :, 3 * QB : 4 * QB], in_=xt_d[:, :, 3 * QB : 4 * QB]
            )
            nc.sync.dma_start(out=wo_sb[:], in_=wo_d[:, :, :])

        bq_sb = auxf_sb[:, 0:2]
        bk_sb = auxf_sb[:, 2:4]
        bv_sb = auxf_sb[:, 4 : 4 + HPC * DH]

        def qk_chunk(p, ch):
            # Q^T and K^T chunk ch for pair p: [dh-pair (128), 512 q]
            for dst, kp, bias in (
                (qt_sb, 2 * p, bq_sb),
                (kt_sb, 2 * p + 1, bk_sb),
            ):
                qp = pj.tile([P, QB], F32, name="qp", tag="pj")
                if qk_fp8:
                    # fp8 DoubleRow: 2 dm-tiles (planes) per pass
                    for t2 in range(NDM // 2):
                        nc.tensor.matmul(
                            qp[:],
                            wqk_sb[:, kp, 2 * t2 : 2 * t2 + 2, :],
                            xt8_sb[:, 2 * t2 : 2 * t2 + 2,
                                   ch * QB : (ch + 1) * QB],
                            start=(t2 == 0),
                            stop=(t2 == NDM // 2 - 1),
                            perf_mode=mybir.MatmulPerfMode.DoubleRow,
                        )
                else:
                    for t in range(NDM):
                        _mm(
                            nc,
                            qp[:],
                            wqk_sb[:, kp, t, :],
                            xt_sb[:, t, ch * QB : (ch + 1) * QB],
                            start=(t == 0),
                            stop=(t == NDM - 1),
                        )
                nc.vector.tensor_scalar_add(
                    dst[p][:, ch * QB : (ch + 1) * QB],
                    qp[:],
                    bias[:, p : p + 1],
                )

        def v_tile(st):
            # V: [seq-tile, 4 heads dh] -> per-pair tiles
            vp = pj.tile([P, QB], F32, name="vp", tag="pj")
            for t in range(NDM):
                _mm(
                    nc,
                    vp[:, 0 : HPC * DH],
                    xt_sb[:, t, st * P : (st + 1) * P],
                    wv_sb[:, t * HPC * DH : (t + 1) * HPC * DH],
                    start=(t == 0),
                    stop=(t == NDM - 1),
                )
            for p in range(2):
                nc.vector.tensor_add(
                    v_sb[p][:, st, :],
                    vp[:, p * P : (p + 1) * P],
                    bv_sb[:, p * P : (p + 1) * P],
                )

        # scores arrive scaled by WSCALE^2 in fp8 mode; fold into the exp scale
        EXP_SCALE = 0.125 / (WSCALE * WSCALE if qk_fp8 else 1.0)

        def attn_qb(p, qb):
            q0 = qb * QB
            nk = (qb + 1) * (QB // P)  # k tiles in causal range
            zps = zp.tile([P, QB], F32, name="zps", tag="z")
            dnb = dp.tile([P, QB], F32, name="dnb", tag="d")

            def pv_dnb(pA, pB, kg, pA01, pB01):
                # PV (column-packed heads) + softmax denominators: the
                # ones-matmul sums the gpsimd-presummed P planes over k AND
                # broadcasts over the 64 rows of each head half; one dnb
                # pass per k-group instead of one per k-tile.
                nkg = nk // 2
                c0g = max(kg * 2 * P - q0, 0)
                ins = []
                for j in range(2):
                    kt = kg * 2 + j
                    c0 = max(kt * P - q0, 0)
                    ins += [
                        _mm(
                            nc, zps[0:64, c0:QB], v_sb[p][:, kt, 0:64],
                            pA[:, j, c0:QB],
                            start=(kt == 0), stop=(kt == nk - 1), skip=True,
                        ),
                        _mm(
                            nc, zps[64:P, c0:QB], v_sb[p][:, kt, 64:P],
                            pB[:, j, c0:QB],
                            start=(kt == 0), stop=(kt == nk - 1), skip=True,
                        ),
                    ]
                ins += [
                    _mm(
                        nc, dnb[0:64, c0g:QB], ones64[:], pA01[:, c0g:QB],
                        start=(kg == 0), stop=(kg == nkg - 1), skip=True,
                    ),
                    _mm(
                        nc, dnb[64:P, c0g:QB], ones64[:], pB01[:, c0g:QB],
                        start=(kg == 0), stop=(kg == nkg - 1), skip=True,
                    ),
                ]
                _chain(ins)

            for kg in range(nk // 2):
                # offs[j]: first valid q column of k-tile kg*2+j
                offs = [kg * 2 * P + j * P - q0 for j in range(2)]
                band = offs[0] >= 0
                deep = band and offs[0] >= 2 * P  # o=1 band k-group
                sA = sp.tile([P, 2, QB], F32, name="sA", tag="s")
                sB = sp.tile([P, 2, QB], F32, name="sB", tag="s")
                for j in range(2):
                    # band k-tiles only compute scores for columns with any
                    # unmasked row; the skipped region holds stale (bounded)
                    # psum that exp+mask neutralizes. The very first k-group
                    # computes everything -- its psum slots are uninitialized.
                    first = qb == 0 and kg == 0
                    c0 = max(offs[j], 0) if band and not first else 0
                    _chain([
                        _mm(
                            nc,
                            stile[:, j, c0:QB],
                            kt_sb[p][rows, (kg * 2 + j) * P : (kg * 2 + j + 1) * P],
                            qt_sb[p][rows, q0 + c0 : q0 + QB],
                            start=True,
                            stop=True,
                        )
                        for rows, stile in ((slice(0, 64), sA), (slice(64, P), sB))
                    ])
                pA = ppool.tile([P, 2, QB], BF16, name="pA", tag="pt")
                pB = ppool.tile([P, 2, QB], BF16, name="pB", tag="pt")
                # exp(S/sqrt(dh)); scale folded into ACT
                if deep:
                    # left of offs[0] is fully masked for both j: zero it and
                    # exp the rest in one shot (j1's leading slice is stale
                    # but bounded; the band mask zeroes it below)
                    for px, sx in ((pA, sA), (pB, sB)):
                        nc.vector.memset(px[:, :, 0 : offs[0]], 0.0)
                        nc.scalar.activation(
                            px[:, :, offs[0] : QB],
                            sx[:, :, offs[0] : QB],
                            mybir.ActivationFunctionType.Exp,
                            scale=EXP_SCALE,
                        )
                else:
                    nc.scalar.activation(
                        pA[:], sA[:], mybir.ActivationFunctionType.Exp,
                        scale=EXP_SCALE,
                    )
                    nc.scalar.activation(
                        pB[:], sB[:], mybir.ActivationFunctionType.Exp,
                        scale=EXP_SCALE,
                    )
                if band:
                    # causal mask: multiply diagonal-band P tiles by 0/1
                    o = offs[0] // (2 * P)
                    nc.vector.tensor_mul(pA[:], pA[:], bandm_sb[:, o, :])
                    nc.vector.tensor_mul(pB[:], pB[:], bandm_sb[:, o, :])
                # presummed P planes (gpsimd, idle engine) halve the dnb
                # matmul passes
                c0g = max(offs[0], 0)
                pA01 = ppool.tile([P, QB], BF16, name="pA01", tag="ps")
                pB01 = ppool.tile([P, QB], BF16, name="pB01", tag="ps")
                nc.vector.tensor_add(
                    pA01[:, c0g:QB], pA[:, 0, c0g:QB], pA[:, 1, c0g:QB]
                )
                nc.vector.tensor_add(
                    pB01[:, c0g:QB], pB[:, 0, c0g:QB], pB[:, 1, c0g:QB]
                )
                pv_dnb(pA, pB, kg, pA01, pB01)

            bcs = bcpool.tile([P, QB], F32, name="bcs", tag="bcs")
            bcr = bcpool.tile([P, QB], F32, name="bcr", tag="bcr")
            nc.vector.reciprocal_approx_accurate(
                out=bcr[:], in_=dnb[:], scratch=bcs[:]
            )
            nc.vector.tensor_mul(zt_sb[p][:, q0 : q0 + QB], zps[:], bcr[:])

        def o_chunk(st, tail=False):
            # O partial rows st*128..: contraction over both pairs' Z^T.
            # In the tail (after the last exp) the scalar engine is idle, so
            # split the copy work across scalar+vector and DMA each half
            # as soon as it lands.
            ot = ost.tile([P, DM], BF16, name="ot", tag="ot")
            for nn in range(2):
                ops = pj.tile([P, QB], F32, name="ops", tag="pj")
                for pp in range(2):
                    _mm(
                        nc,
                        ops[:],
                        zt_sb[pp][:, st * P : (st + 1) * P],
                        wo_sb[:, pp, nn * QB : (nn + 1) * QB],
                        start=(pp == 0),
                        stop=(pp == 1),
                    )
                if tail and nn == 1:
                    nc.scalar.copy(ot[:, nn * QB : (nn + 1) * QB], ops[:])
                else:
                    nc.vector.tensor_copy(ot[:, nn * QB : (nn + 1) * QB], ops[:])
                if tail:
                    nc.sync.dma_start(
                        out=out_d[st * P : (st + 1) * P,
                                  nn * QB : (nn + 1) * QB],
                        in_=ot[:, nn * QB : (nn + 1) * QB],
                    )
            if not tail:
                nc.sync.dma_start(
                    out=out_d[st * P : (st + 1) * P, :], in_=ot[:]
                )

        # ---- pipelined emission ----
        # Phase A: pair-0 projections stream in with the input DMAs;
        # attention(0, qb) follows its chunk.
        for ch in range(NQB):
            qk_chunk(0, ch)
            for st in range(4 * ch, 4 * ch + 4):
                v_tile(st)
            attn_qb(0, ch)
        # Phase B: pair-1 projection chunks interleave with attention(1);
        # O-projection chunks lag one q-block behind as PE filler.
        for ch in range(NQB):
            qk_chunk(1, ch)
            attn_qb(1, ch)
            if ch >= 1:
                for st in range(4 * (ch - 1), 4 * ch):
                    o_chunk(st)
        for st in range(12, 16):
            o_chunk(st, tail=True)

    nc.compile()
    _PROGRAM_CACHE[qk_fp8] = nc
    return nc


def make_in_maps(
    normalized_resid_pre, W_Q, W_K, W_V, W_O, b_Q, b_K, b_V, b_O,
    qk_fp8=QK_FP8,
):
    """Shard + prearrange the full inputs into per-core input maps."""
    import ml_dtypes  # noqa: F401  (registers bfloat16 with numpy)

    bf16 = np.dtype("bfloat16")
    fp8 = np.dtype(ml_dtypes.float8_e4m3)  # TRN fp8e4 (max 240) variant

    x = np.asarray(normalized_resid_pre, dtype=np.float32)
    W_Q = np.asarray(W_Q, dtype=np.float32)
    W_K = np.asarray(W_K, dtype=np.float32)
    W_V = np.asarray(W_V, dtype=np.float32)
    W_O = np.asarray(W_O, dtype=np.float32)
    b_Q = np.asarray(b_Q, dtype=np.float32)
    b_K = np.asarray(b_K, dtype=np.float32)
    b_V = np.asarray(b_V, dtype=np.float32)

    # xt[p, t, s] = x[b][s, t*128+p]
    xts = []
    for b in range(B):
        xt = np.ascontiguousarray(
            x[b].T.reshape(NDM, P, S).transpose(1, 0, 2)
        ).astype(bf16)
        xts.append(xt)

    # additive causal band masks at k-group granularity: variant o covers the
    # two k-tiles at q-block offsets (2o*128, (2o+1)*128)
    kp = np.arange(P)[:, None]
    qc = np.arange(QB)[None, :]
    bandm = np.stack(
        [
            np.concatenate(
                [
                    np.where(qc < (2 * o + j) * P + kp,
                             np.float32(0.0), np.float32(1.0))
                    for j in range(2)
                ],
                axis=1,
            )
            for o in range(2)
        ],
        axis=1,
    ).astype(bf16)  # [P, 2, 2*QB]

    in_maps = []
    for c in range(NCORES):
        b = c // (NCORES // B)
        heads = [HPC * (c % (NCORES // B)) + i for i in range(HPC)]
        wq_cat = np.concatenate([W_Q[h] for h in heads], axis=1)  # [DM, 256]
        wk_cat = np.concatenate([W_K[h] for h in heads], axis=1)
        wv_cat = np.concatenate([W_V[h] for h in heads], axis=1)
        wo_cat = np.concatenate([W_O[h] for h in heads], axis=0)  # [256, DM]

        # wqk[p, kp, t, cc] = W[t*128+p, pair*128 + cc]
        def pack_w(wcat, pair):
            wp = wcat[:, pair * P : (pair + 1) * P]         # [DM, 128]
            return wp.reshape(NDM, P, P).transpose(1, 0, 2)  # [P, NDM, P]

        wqk = np.stack(
            [pack_w(wq_cat, 0), pack_w(wk_cat, 0),
             pack_w(wq_cat, 1), pack_w(wk_cat, 1)],
            axis=1,
        )  # [P, 4, NDM, P] fp32
        if qk_fp8:
            wqk = (wqk * np.float32(WSCALE)).astype(fp8)
        else:
            wqk = wqk.astype(bf16)

        wv = (
            wv_cat.reshape(NDM, P, HPC * DH)
            .transpose(1, 0, 2)
            .reshape(P, NDM * HPC * DH)
            .astype(bf16)
        )
        wo = (
            wo_cat.reshape(2, P, DM).transpose(1, 0, 2).astype(bf16)
        )  # [P, 2, DM]

        # in fp8 mode Q''/K'' carry a WSCALE factor, so biases scale too
        bsc = np.float32(WSCALE if qk_fp8 else 1.0)
        auxf = np.zeros((P, 4 + HPC * DH), dtype=np.float32)
        auxf[:, 0] = np.concatenate([b_Q[heads[0]], b_Q[heads[1]]]) * bsc
        auxf[:, 1] = np.concatenate([b_Q[heads[2]], b_Q[heads[3]]]) * bsc
        auxf[:, 2] = np.concatenate([b_K[heads[0]], b_K[heads[1]]]) * bsc
        auxf[:, 3] = np.concatenate([b_K[heads[2]], b_K[heads[3]]]) * bsc
        auxf[:, 4:] = np.concatenate([b_V[h] for h in heads])[None, :]

        im = {
            "xt": np.ascontiguousarray(xts[b].reshape(P, NDM, S)),
            "wqk": np.ascontiguousarray(wqk),
            "wv": wv,
            "wo": np.ascontiguousarray(wo),
            "auxf": auxf,
            "bandm": np.ascontiguousarray(bandm),
        }
        if qk_fp8:
            im["xt8"] = np.ascontiguousarray(xts[b].astype(np.float32)).astype(
                fp8
            ).reshape(P, NDM, S)
        in_maps.append(im)
    return in_maps


def kernel(normalized_resid_pre, W_Q, W_K, W_V, W_O, b_Q, b_K, b_V, b_O):
    global LAST_RESULTS
    nc = build_program()
    in_maps = make_in_maps(
        normalized_resid_pre, W_Q, W_K, W_V, W_O, b_Q, b_K, b_V, b_O
    )
    trace = os.environ.get("ATTN_TRACE", "0") == "1"
    res = run_bass_kernel_spmd(nc, in_maps, list(range(NCORES)), trace=trace)
    LAST_RESULTS = res

    b_O = np.asarray(b_O, dtype=np.float32)
    parts = [
        np.asarray(res.results[c]["out"], dtype=np.float64) for c in range(NCORES)
    ]
    npc = NCORES // B  # cores per batch
    out = np.stack(
        [sum(parts[b * npc : (b + 1) * npc]) + b_O for b in range(B)]
    )
    return out.astype(np.float32)


# revision 36
# speedup vs baseline: 1.1565x; 1.0532x over previous
"""Trainium2 Bass kernel for causal multi-head attention (dense transformer block).

Problem (hardcoded): x [2, 2048, 1024], 16 heads x 64 dh, causal attention,
fp32 I/O. Sharding: 8 cores = 2 batches x 4 head-groups. Each core computes 4
heads for one batch plus a partial output projection [2048, 1024] (bf16); the
host sums the 4 partials per batch and adds b_O.

Everything on-device is computed in "transposed" orientation so no transposes
are needed anywhere:
  x^T (host-pretransposed)  ->  Q^T, K^T [dh, s] and V [s, dh] via matmuls
  S^T[k, q] = K Q^T         ->  P^T = exp(S^T / 8) (causal-masked pre-exp)
  Z^T[dh, q] = V^T P^T      ->  normalized by column sums (ones-matmul)
  O[s, :]   = (Z^T)^T W_O   (Z^T is directly the lhsT of the O-projection)

Heads are processed in pairs: QK^T packs 2 heads in row-groups (0-63 / 64-127)
of the PE array, PV packs 2 heads in column-groups -- both run concurrently.

Schedule: the whole kernel is software-pipelined in emission order so the
scalar engine (softmax exp, the per-core floor at ~84us incl. per-inst
overhead) overlaps projection/O-projection PE work:
  warmup (PE HAM cold-clock + ACT exp-table load) during the input DMA wait
  for ch: qk_proj(pair0, ch) + v_proj(4 tiles); attention(pair0, qb=ch)
  for ch: qk_proj(pair1, ch); attention(pair1, qb=ch); O-proj chunks (lag 1 qb)
PSUM: scores 2x2 banks, zps 1, dnb 1, proj/O 2 = 8 banks.

Q/K projections run as fp8e4 DoubleRow matmuls (2 dm-planes per pass, ~2x);
V / P*V / O stay bf16 -- their quantization noise lands directly in the
output, while Q/K noise averages out through the softmax. Softmax
denominators use DVE-presummed P planes so the dnb ones-matmul costs one
pass per k-group. Causal masking is banded: fully-masked column ranges are
never computed (c0 trims), diagonal bands get exp + 0/1-mask-multiply.
"""

import os
from contextlib import ExitStack

import numpy as np

import concourse.tile as tile
from concourse import bacc, mybir
from concourse.bass_utils import run_bass_kernel_spmd

# problem constants
B, S, DM, H, DH = 2, 2048, 1024, 16, 64
P = 128          # partitions
QB = 512         # q block (matmul moving free dim)
NKT = S // P     # 16 k tiles
NQB = S // QB    # 4 q blocks
NDM = DM // P    # 8 d_model tiles
HPC = 4          # heads per core
NCORES = 8
NWARM = 12       # PE warmup matmuls; cover the HAM cold window + DMA wait

F32 = mybir.dt.float32
BF16 = mybir.dt.bfloat16
FP8 = mybir.dt.float8e4

# fp8 DoubleRow Q/K projections: W_Q/W_K and x are quantized to fp8e4 on the
# host (weights pre-scaled by 64 so they clear the fp8 subnormal range; the
# 1/64^2 un-scale is folded into the softmax exp scale). Q/K quantization
# noise is random across the contraction and averages out through the
# softmax, unlike V/W_O noise which lands directly in the output.
QK_FP8 = os.environ.get("ATTN_QK_FP8", "1") == "1"
WSCALE = 64.0

_PROGRAM_CACHE = {}
LAST_RESULTS = None  # BassKernelResults of the most recent run (for test.py)


def _mm(nc, out, lhsT, rhs, start, stop, skip=False):
    # skip_group_check: the sim's psum-group tracker doesn't distinguish
    # partition ranges; our concurrent groups in one bank are partition-disjoint
    # (rows 0-63 vs 64-127), which the per-partition zeroing model handles.
    return nc.tensor.matmul(
        out, lhsT, rhs, start=start, stop=stop, skip_group_check=skip
    )


def _chain(insts):
    """Ordering-only PE edges so matmuls alternating between row/column
    groups stay adjacent and run concurrently on the array."""
    from concourse.tile import add_dep_helper

    for a, b in zip(insts[1:], insts):
        add_dep_helper(a.ins, b.ins, sync=False, reason="pack-pair order")


def build_program(qk_fp8=QK_FP8):
    """Build the single-core SPMD Bass program (same program on all 8 cores)."""
    if qk_fp8 in _PROGRAM_CACHE:
        return _PROGRAM_CACHE[qk_fp8]

    nc = bacc.Bacc(
        "TRN2", target_bir_lowering=False, debug=False, num_devices=NCORES
    )

    # ---- DRAM I/O (per-core shards, prearranged on host in SBUF layout) ----
    # xt:   [p, t, s]        = x^T[t*128+p, s]
    # wqk:  [p, kp, t*128+c] kp in (wq-p0, wk-p0, wq-p1, wk-p1), = W[t*128+p, pair-col c]
    # wv:   [p, t*256+c]     = W_V[t*128+p, c]   (c over all 4 heads)
    # wo:   [p, pair, c]     = W_O_cat[pair*128+p, c]
    # auxf: [p, 0:2]=bq pair cols, [2:4]=bk, [4:260]=bv row (bcast over p)
    # bandm:[p, o, c]        0/1 causal band masks
    WDT = FP8 if qk_fp8 else BF16
    xt_d = nc.dram_tensor("xt", [P, NDM, S], BF16, kind="ExternalInput")
    if qk_fp8:
        xt8_d = nc.dram_tensor("xt8", [P, NDM, S], FP8, kind="ExternalInput")
    wqk_d = nc.dram_tensor("wqk", [P, 4, NDM, P], WDT, kind="ExternalInput")
    wv_d = nc.dram_tensor("wv", [P, NDM * HPC * DH], BF16, kind="ExternalInput")
    wo_d = nc.dram_tensor("wo", [P, 2, DM], BF16, kind="ExternalInput")
    auxf_d = nc.dram_tensor("auxf", [P, 4 + HPC * DH], F32, kind="ExternalInput")
    bandm_d = nc.dram_tensor("bandm", [P, 2, 2 * QB], BF16, kind="ExternalInput")
    out_d = nc.dram_tensor("out", [S, DM], BF16, kind="ExternalOutput")

    with tile.TileContext(nc) as tc, ExitStack() as ctx:
        const = ctx.enter_context(tc.tile_pool(name="const", bufs=1))
        persist = ctx.enter_context(tc.tile_pool(name="persist", bufs=1))

        # ---- SBUF persistent tensors ----
        xt_sb = persist.tile([P, NDM, S], BF16, name="xt_sb", tag="xt")
        if qk_fp8:
            xt8_sb = persist.tile([P, NDM, S], FP8, name="xt8_sb", tag="xt8")
        wqk_sb = persist.tile([P, 4, NDM, P], WDT, name="wqk_sb", tag="wqk")
        wv_sb = persist.tile([P, NDM * HPC * DH], BF16, name="wv_sb", tag="wv")
        wo_sb = persist.tile([P, 2, DM], BF16, name="wo_sb", tag="wo")
        auxf_sb = persist.tile([P, 4 + HPC * DH], F32, name="auxf_sb", tag="auxf")
        bandm_sb = persist.tile([P, 2, 2 * QB], BF16, name="bandm_sb", tag="bandm")
        qt_sb = [
            persist.tile([P, S], BF16, name=f"qt{p}", tag=f"qt{p}") for p in range(2)
        ]
        kt_sb = [
            persist.tile([P, S], BF16, name=f"kt{p}", tag=f"kt{p}") for p in range(2)
        ]
        v_sb = [
            persist.tile([P, NKT, P], BF16, name=f"v{p}", tag=f"v{p}")
            for p in range(2)
        ]
        zt_sb = [
            persist.tile([P, S], BF16, name=f"zt{p}", tag=f"zt{p}") for p in range(2)
        ]
        ones64 = const.tile([P, 64], BF16, name="ones64", tag="ones64")
        warm_in = const.tile([P, QB], BF16, name="warm_in", tag="warm_in")
        warm_out = const.tile([P, 8], F32, name="warm_out", tag="warm_out")

        # ---- PSUM pools: 2*2 (scores) + 1 (z) + 1 (d) + 2 (proj/O) = 8 banks
        sp = ctx.enter_context(tc.tile_pool(name="sp", bufs=2, space="PSUM"))
        zp = ctx.enter_context(tc.tile_pool(name="zp", bufs=1, space="PSUM"))
        dp = ctx.enter_context(tc.tile_pool(name="dp", bufs=1, space="PSUM"))
        pj = ctx.enter_context(tc.tile_pool(name="pj", bufs=2, space="PSUM"))

        ppool = ctx.enter_context(tc.tile_pool(name="ppool", bufs=8))
        bcpool = ctx.enter_context(tc.tile_pool(name="bcpool", bufs=2))
        ost = ctx.enter_context(tc.tile_pool(name="ost", bufs=3))

        # ---- warmup: runs during the input DMA wait ----
        nc.gpsimd.memset(ones64[:], 1.0)
        nc.gpsimd.memset(warm_in[:], 0.0)
        # preload the exp table set (~2.7us) before the first real exp
        nc.scalar.activation(
            warm_out[:], warm_in[:, 0:8], mybir.ActivationFunctionType.Exp,
            scale=1.0,
        )
        for w in range(NWARM):
            wps = pj.tile([P, QB], F32, name="wps", tag="pj")
            _mm(nc, wps[0:64, :], ones64[:], warm_in[:], start=True, stop=True)

        # ---- input DMAs (sync queue), in compute-readiness order ----
        if qk_fp8:
            # Q/K path reads fp8 copies; bf16 x feeds only the V projection.
            # First x chunk split in half so the first matmuls start sooner.
            nc.sync.dma_start(out=wqk_sb[:], in_=wqk_d[:, :, :, :])
            nc.sync.dma_start(out=xt8_sb[:, 0:4, 0:QB], in_=xt8_d[:, 0:4, 0:QB])
            nc.sync.dma_start(out=xt8_sb[:, 4:8, 0:QB], in_=xt8_d[:, 4:8, 0:QB])
            nc.sync.dma_start(out=auxf_sb[:], in_=auxf_d[:, :])
            nc.sync.dma_start(out=xt_sb[:, :, 0:QB], in_=xt_d[:, :, 0:QB])
            nc.sync.dma_start(out=wv_sb[:], in_=wv_d[:, :])
            nc.sync.dma_start(out=bandm_sb[:], in_=bandm_d[:, :, :])
            for ch in range(1, NQB):
                c0, c1 = ch * QB, (ch + 1) * QB
                nc.sync.dma_start(out=xt8_sb[:, :, c0:c1], in_=xt8_d[:, :, c0:c1])
                nc.sync.dma_start(out=xt_sb[:, :, c0:c1], in_=xt_d[:, :, c0:c1])
            nc.sync.dma_start(out=wo_sb[:], in_=wo_d[:, :, :])
        else:
            nc.sync.dma_start(out=wqk_sb[:, 0], in_=wqk_d[:, 0])   # wq pair0
            nc.sync.dma_start(out=xt_sb[:, :, 0:QB], in_=xt_d[:, :, 0:QB])
            nc.sync.dma_start(out=auxf_sb[:], in_=auxf_d[:, :])
            nc.sync.dma_start(out=wqk_sb[:, 1], in_=wqk_d[:, 1])   # wk pair0
            nc.sync.dma_start(out=wv_sb[:], in_=wv_d[:, :])
            nc.sync.dma_start(out=bandm_sb[:], in_=bandm_d[:, :, :])
            nc.sync.dma_start(
                out=xt_sb[:, :, QB : 2 * QB], in_=xt_d[:, :, QB : 2 * QB]
            )
            nc.sync.dma_start(out=wqk_sb[:, 2], in_=wqk_d[:, 2])   # wq pair1
            nc.sync.dma_start(out=wqk_sb[:, 3], in_=wqk_d[:, 3])   # wk pair1
            nc.sync.dma_start(
                out=xt_sb[:, :, 2 * QB : 3 * QB], in_=xt_d[:, :, 2 * QB : 3 * QB]
            )
            nc.sync.dma_start(
                out=xt_sb[:, :, 3 * QB : 4 * QB], in_=xt_d[:, :, 3 * QB : 4 * QB]
            )
            nc.sync.dma_start(out=wo_sb[:], in_=wo_d[:, :, :])

        bq_sb = auxf_sb[:, 0:2]
        bk_sb = auxf_sb[:, 2:4]
        bv_sb = auxf_sb[:, 4 : 4 + HPC * DH]

        def qk_half(p, ch, which):
            # Q^T (which=0) or K^T (which=1) chunk ch for pair p:
            # [dh-pair (128), 512 q] -- one schedulable PE work unit
            dst = qt_sb if which == 0 else kt_sb
            kp = 2 * p + which
            bias = bq_sb if which == 0 else bk_sb
            if True:
                qp = pj.tile([P, QB], F32, name="qp", tag="pj")
                if qk_fp8:
                    # fp8 DoubleRow: 2 dm-tiles (planes) per pass
                    for t2 in range(NDM // 2):
                        nc.tensor.matmul(
                            qp[:],
                            wqk_sb[:, kp, 2 * t2 : 2 * t2 + 2, :],
                            xt8_sb[:, 2 * t2 : 2 * t2 + 2,
                                   ch * QB : (ch + 1) * QB],
                            start=(t2 == 0),
                            stop=(t2 == NDM // 2 - 1),
                            perf_mode=mybir.MatmulPerfMode.DoubleRow,
                        )
                else:
                    for t in range(NDM):
                        _mm(
                            nc,
                            qp[:],
                            wqk_sb[:, kp, t, :],
                            xt_sb[:, t, ch * QB : (ch + 1) * QB],
                            start=(t == 0),
                            stop=(t == NDM - 1),
                        )
                nc.vector.tensor_scalar_add(
                    dst[p][:, ch * QB : (ch + 1) * QB],
                    qp[:],
                    bias[:, p : p + 1],
                )

        def qk_chunk(p, ch):
            qk_half(p, ch, 0)
            qk_half(p, ch, 1)

        def v_tile(st):
            # V: [seq-tile, 4 heads dh] -> per-pair tiles
            vp = pj.tile([P, QB], F32, name="vp", tag="pj")
            for t in range(NDM):
                _mm(
                    nc,
                    vp[:, 0 : HPC * DH],
                    xt_sb[:, t, st * P : (st + 1) * P],
                    wv_sb[:, t * HPC * DH : (t + 1) * HPC * DH],
                    start=(t == 0),
                    stop=(t == NDM - 1),
                )
            for p in range(2):
                nc.vector.tensor_add(
                    v_sb[p][:, st, :],
                    vp[:, p * P : (p + 1) * P],
                    bv_sb[:, p * P : (p + 1) * P],
                )

        # scores arrive scaled by WSCALE^2 in fp8 mode; fold into the exp scale
        EXP_SCALE = 0.125 / (WSCALE * WSCALE if qk_fp8 else 1.0)

        def attn_qb(p, qb, fillers=()):
            """fillers: emission closures (small PE work units) woven one per
            k-group into the gaps where the PE would otherwise wait for the
            scalar engine's exp of that k-group."""
            fillers = list(fillers)
            q0 = qb * QB
            nk = (qb + 1) * (QB // P)  # k tiles in causal range
            zps = zp.tile([P, QB], F32, name="zps", tag="z")
            dnb = dp.tile([P, QB], F32, name="dnb", tag="d")

            def pv_dnb(pA, pB, kg, pA01, pB01):
                # PV (column-packed heads) + softmax denominators: the
                # ones-matmul sums the gpsimd-presummed P planes over k AND
                # broadcasts over the 64 rows of each head half; one dnb
                # pass per k-group instead of one per k-tile.
                nkg = nk // 2
                c0g = max(kg * 2 * P - q0, 0)
                ins = []
                for j in range(2):
                    kt = kg * 2 + j
                    c0 = max(kt * P - q0, 0)
                    ins += [
                        _mm(
                            nc, zps[0:64, c0:QB], v_sb[p][:, kt, 0:64],
                            pA[:, j, c0:QB],
                            start=(kt == 0), stop=(kt == nk - 1), skip=True,
                        ),
                        _mm(
                            nc, zps[64:P, c0:QB], v_sb[p][:, kt, 64:P],
                            pB[:, j, c0:QB],
                            start=(kt == 0), stop=(kt == nk - 1), skip=True,
                        ),
                    ]
                ins += [
                    _mm(
                        nc, dnb[0:64, c0g:QB], ones64[:], pA01[:, c0g:QB],
                        start=(kg == 0), stop=(kg == nkg - 1), skip=True,
                    ),
                    _mm(
                        nc, dnb[64:P, c0g:QB], ones64[:], pB01[:, c0g:QB],
                        start=(kg == 0), stop=(kg == nkg - 1), skip=True,
                    ),
                ]
                _chain(ins)

            for kg in range(nk // 2):
                # offs[j]: first valid q column of k-tile kg*2+j
                offs = [kg * 2 * P + j * P - q0 for j in range(2)]
                band = offs[0] >= 0
                deep = band and offs[0] >= 2 * P  # o=1 band k-group
                sA = sp.tile([P, 2, QB], F32, name="sA", tag="s")
                sB = sp.tile([P, 2, QB], F32, name="sB", tag="s")
                for j in range(2):
                    # band k-tiles only compute scores for columns with any
                    # unmasked row; the skipped region holds stale (bounded)
                    # psum that exp+mask neutralizes. The very first k-group
                    # computes everything -- its psum slots are uninitialized.
                    first = qb == 0 and kg == 0
                    c0 = max(offs[j], 0) if band and not first else 0
                    _chain([
                        _mm(
                            nc,
                            stile[:, j, c0:QB],
                            kt_sb[p][rows, (kg * 2 + j) * P : (kg * 2 + j + 1) * P],
                            qt_sb[p][rows, q0 + c0 : q0 + QB],
                            start=True,
                            stop=True,
                        )
                        for rows, stile in ((slice(0, 64), sA), (slice(64, P), sB))
                    ])
                pA = ppool.tile([P, 2, QB], BF16, name="pA", tag="pt")
                pB = ppool.tile([P, 2, QB], BF16, name="pB", tag="pt")
                # exp(S/sqrt(dh)); scale folded into ACT
                if deep:
                    # left of offs[0] is fully masked for both j: zero it and
                    # exp the rest in one shot (j1's leading slice is stale
                    # but bounded; the band mask zeroes it below)
                    for px, sx in ((pA, sA), (pB, sB)):
                        nc.vector.memset(px[:, :, 0 : offs[0]], 0.0)
                        nc.scalar.activation(
                            px[:, :, offs[0] : QB],
                            sx[:, :, offs[0] : QB],
                            mybir.ActivationFunctionType.Exp,
                            scale=EXP_SCALE,
                        )
                else:
                    nc.scalar.activation(
                        pA[:], sA[:], mybir.ActivationFunctionType.Exp,
                        scale=EXP_SCALE,
                    )
                    nc.scalar.activation(
                        pB[:], sB[:], mybir.ActivationFunctionType.Exp,
                        scale=EXP_SCALE,
                    )
                if band:
                    # causal mask: multiply diagonal-band P tiles by 0/1
                    o = offs[0] // (2 * P)
                    nc.vector.tensor_mul(pA[:], pA[:], bandm_sb[:, o, :])
                    nc.vector.tensor_mul(pB[:], pB[:], bandm_sb[:, o, :])
                # presummed P planes (gpsimd, idle engine) halve the dnb
                # matmul passes
                c0g = max(offs[0], 0)
                pA01 = ppool.tile([P, QB], BF16, name="pA01", tag="ps")
                pB01 = ppool.tile([P, QB], BF16, name="pB01", tag="ps")
                nc.vector.tensor_add(
                    pA01[:, c0g:QB], pA[:, 0, c0g:QB], pA[:, 1, c0g:QB]
                )
                nc.vector.tensor_add(
                    pB01[:, c0g:QB], pB[:, 0, c0g:QB], pB[:, 1, c0g:QB]
                )
                # PE filler while the scalar engine runs this k-group's exp
                if kg < len(fillers):
                    fillers[kg]()
                pv_dnb(pA, pB, kg, pA01, pB01)

            bcs = bcpool.tile([P, QB], F32, name="bcs", tag="bcs")
            bcr = bcpool.tile([P, QB], F32, name="bcr", tag="bcr")
            nc.vector.reciprocal_approx_accurate(
                out=bcr[:], in_=dnb[:], scratch=bcs[:]
            )
            nc.vector.tensor_mul(zt_sb[p][:, q0 : q0 + QB], zps[:], bcr[:])

        def o_chunk(st, tail=False):
            # O partial rows st*128..: contraction over both pairs' Z^T.
            # In the tail (after the last exp) the scalar engine is idle, so
            # split the copy work across scalar+vector and DMA each half
            # as soon as it lands.
            ot = ost.tile([P, DM], BF16, name="ot", tag="ot")
            for nn in range(2):
                ops = pj.tile([P, QB], F32, name="ops", tag="pj")
                for pp in range(2):
                    _mm(
                        nc,
                        ops[:],
                        zt_sb[pp][:, st * P : (st + 1) * P],
                        wo_sb[:, pp, nn * QB : (nn + 1) * QB],
                        start=(pp == 0),
                        stop=(pp == 1),
                    )
                if tail and nn == 1:
                    nc.scalar.copy(ot[:, nn * QB : (nn + 1) * QB], ops[:])
                else:
                    nc.vector.tensor_copy(ot[:, nn * QB : (nn + 1) * QB], ops[:])
                if tail:
                    nc.sync.dma_start(
                        out=out_d[st * P : (st + 1) * P,
                                  nn * QB : (nn + 1) * QB],
                        in_=ot[:, nn * QB : (nn + 1) * QB],
                    )
            if not tail:
                nc.sync.dma_start(
                    out=out_d[st * P : (st + 1) * P, :], in_=ot[:]
                )

        # ---- pipelined emission ----
        from functools import partial

        # Phase A: pair-0 projections stream in with the input DMAs; V tiles
        # and pair-1 Q/K chunks ride the attention k-group gaps as PE filler.
        qk_chunk(0, 0)
        for st in range(4):
            v_tile(st)
        attn_qb(0, 0)
        for ch in range(1, NQB):
            qk_chunk(0, ch)
            fil = [partial(v_tile, 4 * ch + k) for k in range(4)]
            if ch == NQB - 1:
                fil += [partial(qk_half, 1, 0, 0), partial(qk_half, 1, 0, 1)]
            attn_qb(0, ch, fillers=fil)
        # Phase B: pair-1 Q/K chunks (one block ahead) and O-projection
        # chunks fill attention(1)'s k-group gaps; only the last four O
        # chunks trail attention(1, qb3).
        attn_qb(1, 0, fillers=[
            partial(qk_half, 1, 1, 0), partial(qk_half, 1, 1, 1)])
        attn_qb(1, 1, fillers=[
            partial(qk_half, 1, 2, 0), partial(qk_half, 1, 2, 1),
            partial(o_chunk, 0), partial(o_chunk, 1)])
        attn_qb(1, 2, fillers=[
            partial(qk_half, 1, 3, 0), partial(qk_half, 1, 3, 1),
            partial(o_chunk, 2), partial(o_chunk, 3),
            partial(o_chunk, 4), partial(o_chunk, 5)])
        attn_qb(1, 3, fillers=[partial(o_chunk, st) for st in range(6, 12)])
        for st in range(12, 16):
            o_chunk(st, tail=True)

    nc.compile()
    _PROGRAM_CACHE[qk_fp8] = nc
    return nc


def make_in_maps(
    normalized_resid_pre, W_Q, W_K, W_V, W_O, b_Q, b_K, b_V, b_O,
    qk_fp8=QK_FP8,
):
    """Shard + prearrange the full inputs into per-core input maps."""
    import ml_dtypes  # noqa: F401  (registers bfloat16 with numpy)

    bf16 = np.dtype("bfloat16")
    fp8 = np.dtype(ml_dtypes.float8_e4m3)  # TRN fp8e4 (max 240) variant

    x = np.asarray(normalized_resid_pre, dtype=np.float32)
    W_Q = np.asarray(W_Q, dtype=np.float32)
    W_K = np.asarray(W_K, dtype=np.float32)
    W_V = np.asarray(W_V, dtype=np.float32)
    W_O = np.asarray(W_O, dtype=np.float32)
    b_Q = np.asarray(b_Q, dtype=np.float32)
    b_K = np.asarray(b_K, dtype=np.float32)
    b_V = np.asarray(b_V, dtype=np.float32)

    # xt[p, t, s] = x[b][s, t*128+p]
    xts = []
    for b in range(B):
        xt = np.ascontiguousarray(
            x[b].T.reshape(NDM, P, S).transpose(1, 0, 2)
        ).astype(bf16)
        xts.append(xt)

    # additive causal band masks at k-group granularity: variant o covers the
    # two k-tiles at q-block offsets (2o*128, (2o+1)*128)
    kp = np.arange(P)[:, None]
    qc = np.arange(QB)[None, :]
    bandm = np.stack(
        [
            np.concatenate(
                [
                    np.where(qc < (2 * o + j) * P + kp,
                             np.float32(0.0), np.float32(1.0))
                    for j in range(2)
                ],
                axis=1,
            )
            for o in range(2)
        ],
        axis=1,
    ).astype(bf16)  # [P, 2, 2*QB]

    in_maps = []
    for c in range(NCORES):
        b = c // (NCORES // B)
        heads = [HPC * (c % (NCORES // B)) + i for i in range(HPC)]
        wq_cat = np.concatenate([W_Q[h] for h in heads], axis=1)  # [DM, 256]
        wk_cat = np.concatenate([W_K[h] for h in heads], axis=1)
        wv_cat = np.concatenate([W_V[h] for h in heads], axis=1)
        wo_cat = np.concatenate([W_O[h] for h in heads], axis=0)  # [256, DM]

        # wqk[p, kp, t, cc] = W[t*128+p, pair*128 + cc]
        def pack_w(wcat, pair):
            wp = wcat[:, pair * P : (pair + 1) * P]         # [DM, 128]
            return wp.reshape(NDM, P, P).transpose(1, 0, 2)  # [P, NDM, P]

        wqk = np.stack(
            [pack_w(wq_cat, 0), pack_w(wk_cat, 0),
             pack_w(wq_cat, 1), pack_w(wk_cat, 1)],
            axis=1,
        )  # [P, 4, NDM, P] fp32
        if qk_fp8:
            wqk = (wqk * np.float32(WSCALE)).astype(fp8)
        else:
            wqk = wqk.astype(bf16)

        wv = (
            wv_cat.reshape(NDM, P, HPC * DH)
            .transpose(1, 0, 2)
            .reshape(P, NDM * HPC * DH)
            .astype(bf16)
        )
        wo = (
            wo_cat.reshape(2, P, DM).transpose(1, 0, 2).astype(bf16)
        )  # [P, 2, DM]

        # in fp8 mode Q''/K'' carry a WSCALE factor, so biases scale too
        bsc = np.float32(WSCALE if qk_fp8 else 1.0)
        auxf = np.zeros((P, 4 + HPC * DH), dtype=np.float32)
        auxf[:, 0] = np.concatenate([b_Q[heads[0]], b_Q[heads[1]]]) * bsc
        auxf[:, 1] = np.concatenate([b_Q[heads[2]], b_Q[heads[3]]]) * bsc
        auxf[:, 2] = np.concatenate([b_K[heads[0]], b_K[heads[1]]]) * bsc
        auxf[:, 3] = np.concatenate([b_K[heads[2]], b_K[heads[3]]]) * bsc
        auxf[:, 4:] = np.concatenate([b_V[h] for h in heads])[None, :]

        im = {
            "xt": np.ascontiguousarray(xts[b].reshape(P, NDM, S)),
            "wqk": np.ascontiguousarray(wqk),
            "wv": wv,
            "wo": np.ascontiguousarray(wo),
            "auxf": auxf,
            "bandm": np.ascontiguousarray(bandm),
        }
        if qk_fp8:
            im["xt8"] = np.ascontiguousarray(xts[b].astype(np.float32)).astype(
                fp8
            ).reshape(P, NDM, S)
        in_maps.append(im)
    return in_maps


def kernel(normalized_resid_pre, W_Q, W_K, W_V, W_O, b_Q, b_K, b_V, b_O):
    global LAST_RESULTS
    nc = build_program()
    in_maps = make_in_maps(
        normalized_resid_pre, W_Q, W_K, W_V, W_O, b_Q, b_K, b_V, b_O
    )
    trace = os.environ.get("ATTN_TRACE", "0") == "1"
    res = run_bass_kernel_spmd(nc, in_maps, list(range(NCORES)), trace=trace)
    LAST_RESULTS = res

    b_O = np.asarray(b_O, dtype=np.float32)
    parts = [
        np.asarray(res.results[c]["out"], dtype=np.float64) for c in range(NCORES)
    ]
    npc = NCORES // B  # cores per batch
    out = np.stack(
        [sum(parts[b * npc : (b + 1) * npc]) + b_O for b in range(B)]
    )
    return out.astype(np.float32)
